# revision 1
# baseline (speedup 1.0000x reference)
"""BWGNN-Hetero forward on 8 Trainium2 NeuronCores.

Node-sharded (N/8 nodes per core). Per relation: two polynomial-propagation
steps; segment-sum gathers per-edge source rows (dma_gather, bf16 tables
with 256B row stride) and reduces them with PE matmuls against
on-device-built one-hot selection matrices into 128-node PSUM windows.

The node table is split into 4 QUARTER tables (one per SWDGE queue): local
rows [3200q, 3200(q+1)) of every core are AllGathered into table_q
[8*3200=25600, 128] (int16-indexable). Gather calls for quarter q run on
SWDGE queue q, so descriptor generation is never ring-stalled behind a
single queue and the 4 rings drain concurrently. Each quarter's AllGather
is triggered as soon as the epilogue finishes that quarter's 25 windows,
overlapping the collective with the remaining descriptor generation.

Node state is feature-major and HALF-PACKED: a [128, P/2] tile holds
features of nodes [0,P/2) on partitions 0:64 and of [P/2,P) on partitions
64:128 (matmuls address the upper half via tile_position).

SPMD: one program for all 8 cores; the edge layout is padded to a common
structure (per-(window,quarter) chunk capacity = max over cores) so the
instruction stream is core-invariant while indices/dst data are inputs.
"""

import numpy as np
import ml_dtypes

import concourse.bass as bass
import concourse.mybir as mybir
import concourse.tile as tile
from concourse import ap_utils
from concourse.bass import MemorySpace

N_CORES = 8
H = 64
C_OUT = 2
IN_F = 128
WIN = 128
CALL_MAX = 1024
QUARTERS = 4
SENT = 1024.0
import os
N_SWDGE_QUEUES = int(os.environ.get("K_QUEUES", "4"))


def _qrows(n_local):
    """Padded local rows per quarter (window-aligned)."""
    return -(-n_local // (QUARTERS * WIN)) * WIN

THETAS = np.array([[3.0, -3.0, 0.75],
                   [0.0, 3.0, -1.50],
                   [0.0, 0.0, 0.75]], dtype=np.float32)

BF16 = ml_dtypes.bfloat16
LAST_BUILD = None

# ---------------------------------------------------------------- bir fixes


def _fix_sync_waits(nc, max_waits=1):
    """This walrus build rejects >1 sync-wait per instruction; move excess
    waits onto fresh nops on the same engine queue."""
    counter = [0]
    for fn in nc.m.functions:
        for bb in fn.blocks:
            new_insts = []
            for inst in bb.instructions:
                si = inst.sync_info
                if si is None or not si.on_wait or len(si.on_wait) <= max_waits:
                    new_insts.append(inst)
                    continue
                waits = list(si.on_wait)
                for w in waits[max_waits:]:
                    counter[0] += 1
                    nop = mybir.InstNoOp(name=f"waitsplit_{counter[0]}", ins=[], outs=[])
                    nop.engine = inst.engine
                    nop.sync_info = mybir.SyncInfo(on_wait=[w], on_update=[])
                    nc.register_instruction(nop)
                    new_insts.append(nop)
                inst.sync_info = mybir.SyncInfo(
                    on_wait=waits[:max_waits], on_update=list(si.on_update))
                new_insts.append(inst)
            if len(new_insts) != len(bb.instructions):
                bb.instructions[:] = new_insts


def _insert_library_loads(nc):
    import bass_rust as _bass_rust
    from concourse.library_config import all_libraries, standard
    mask = {}
    for lib in all_libraries:
        for t in lib.instructions:
            mask[t] = mask.get(t, 0) | (1 << lib.index)
    _bass_rust.insert_library_loads(nc, mask, len(all_libraries), standard.index)


def _lower_library_reloads(nc):
    """Rewrite the pseudo library-reload into the raw 64B PSEUDO_INST struct
    this walrus can encode (not sim-executable; call only before HW runs)."""
    import bass_rust as _bass_rust
    isa = nc.isa
    PO = isa.get_enum("NEURON_ISA_TPB_PSEUDO_OPCODE")
    for fn in nc.m.functions:
        for bb in fn.blocks:
            for i, inst in enumerate(bb.instructions):
                if not isinstance(inst, _bass_rust.InstPseudoReloadLibraryIndex):
                    continue
                raw = nc.engines[inst.engine]._isa(
                    isa.Opcode.NEURON_ISA_TPB_OPCODE_PSEUDO_INST,
                    {"pseudo_opcode":
                         PO.NEURON_ISA_TPB_PSEUDO_OPCODE_PSEUDO_LIBRARY_RELOAD_INDEX.value,
                     "lib_index": inst.lib_index},
                    "NEURON_ISA_TPB_PSEUDO_LIBRARY_RELOAD_INDEX_STRUCT",
                    [], [], True)
                raw.engine = inst.engine
                raw.sync_info = inst.sync_info
                nc.register_instruction(raw, overwrite=True)
                bb.instructions[i] = raw


def _dma_gather(gp, out_ap, in_ap, idxs_ap, num_idxs, num_idxs_reg, elem_size,
                elem_step, queue_num=0):
    """dma_gather with the elem_size%256 assert relaxed (row stride must
    still be a 256B multiple; validated on HW)."""
    assert idxs_ap.dtype == mybir.dt.int16
    assert in_ap.dtype == out_ap.dtype
    assert in_ap.space == MemorySpace.DRAM
    assert idxs_ap.space == MemorySpace.SBUF and out_ap.space == MemorySpace.SBUF
    assert ap_utils.ap_is_contiguous(out_ap.ap[1:])
    assert ap_utils.ap_is_contiguous(idxs_ap.ap[1:])
    assert in_ap.ap[-1][1] == out_ap.ap[-1][1] == elem_size
    assert out_ap.ap[0][1] * out_ap.ap[1][1] == ((num_idxs + 127) // 128) * 128
    assert in_ap.ap[0][0] == elem_step
    stride_bytes = elem_step * mybir.dt.size(in_ap.dtype)
    assert stride_bytes % 256 == 0 and stride_bytes // 256 < 256
    _in_ap = gp.lower_ap_dma(in_ap, for_custom_bir_dma=True)
    _idxs_ap = gp.lower_ap(idxs_ap)
    _out_ap = gp.lower_ap(out_ap)
    return gp.add_instruction(
        mybir.InstDMAGatherAnt(
            name=gp.bass.get_next_instruction_name(),
            ins=[*_in_ap, _idxs_ap, gp.lower_val_access(gp.to_reg(num_idxs_reg))],
            outs=[_out_ap],
            transpose=False, num_idxs=num_idxs, elem_size=elem_size,
            stride_bytes_256=stride_bytes // 256, gen_mode=0,
            single_packet=True, queue_num=queue_num, sbuf_tokens_per_rank=0,
            sbuf_free_dim_per_rank=0, sbuf_free_dim_pad_per_rank=0,
            sbuf_byte_offset=0))


# ---------------------------------------------------------------- host plan


def _wrap_idx(idx):
    """[n] -> [128, n/16] int16: idx i at [i%16, i//16], replicated for the
    8 GPSIMD cores across partition groups of 16."""
    n = len(idx)
    assert n % 16 == 0
    w = np.ascontiguousarray(idx.astype(np.int16).reshape(n // 16, 16).T)
    return np.tile(w, (8, 1))


class RelPlan:
    """Common (cross-core) structure + per-core data for one relation."""


def _plan_relation(src, dst, N, n_local):
    n_win = (n_local + WIN - 1) // WIN
    QROWS = _qrows(n_local)

    cores = []
    counts = np.zeros((N_CORES, n_win, QUARTERS), np.int64)
    for c in range(N_CORES):
        lo = c * n_local
        m = (dst >= lo) & (dst < lo + n_local)
        s = src[m]
        d = dst[m] - lo
        cs = s // n_local
        li = s % n_local
        q = li // QROWS
        row = cs * QROWS + (li % QROWS)
        w = d // WIN
        order = np.lexsort((d, w, q))
        row, d, q, w = row[order], d[order], q[order], w[order]
        counts[c] = np.bincount(w * QUARTERS + q,
                                minlength=n_win * QUARTERS).reshape(n_win, QUARTERS)
        cores.append((row, d, q, w))

    cap = np.ceil(counts.max(axis=0) / 128).astype(np.int64)

    chunk_ws = []
    chunk_base = np.zeros((n_win, QUARTERS), np.int64)
    quarter_first_chunk = []
    g = 0
    for q in range(QUARTERS):
        quarter_first_chunk.append(g)
        for w in range(n_win):
            chunk_base[w, q] = g
            for _ in range(cap[w, q]):
                chunk_ws.append((w, q))
                g += 1
    n_chunks = g
    n_slots = n_chunks * 128

    # window-aligned calls: a call covers whole windows of one quarter, so a
    # window's matmuls never straddle two calls of the same quarter (keeps the
    # msg-pool wait chain linear -> no tile-pool deadlock).
    calls = []
    chunk2call = {}
    max_chunks = CALL_MAX // 128
    for q in range(QUARTERS):
        w = 0
        while w < n_win:
            c0 = chunk_base[w, q]
            w2 = w
            take = 0
            while w2 < n_win and take + cap[w2, q] <= max_chunks:
                take += cap[w2, q]
                w2 += 1
            assert take > 0
            cid = len(calls)
            calls.append((int(c0) * 128, int(take) * 128, q))
            for j in range(take):
                chunk2call[int(c0) + j] = (cid, j)
            w = w2

    win_chunks = [[] for _ in range(n_win)]
    for gid, (w, q) in enumerate(chunk_ws):
        win_chunks[w].append(gid)
    gid2ipos = np.zeros(max(1, n_chunks), np.int64)
    ipos = 0
    for w in range(n_win):
        for gid in win_chunks[w]:
            gid2ipos[gid] = ipos
            ipos += 1
    call_first_window = [min((chunk_ws[g][0] for g in
                              range(calls[cid][0] // 128,
                                    calls[cid][0] // 128 + calls[cid][1] // 128)),
                             default=0)
                         for cid in range(len(calls))]

    idx_data = np.zeros((N_CORES, n_slots), np.int64)
    dst_data = np.full((N_CORES, n_slots), -1.0, np.float64)
    for c in range(N_CORES):
        row, d, q, w = cores[c]
        pos = 0
        for qv in range(QUARTERS):
            for wv in range(n_win):
                cnt = counts[c, wv, qv]
                if cnt == 0:
                    continue
                b = chunk_base[wv, qv] * 128
                idx_data[c, b:b + cnt] = row[pos:pos + cnt]
                dst_data[c, b:b + cnt] = d[pos:pos + cnt] - wv * WIN
                pos += cnt
        assert pos == len(row)

    p = RelPlan()
    p.n_win = n_win
    p.n_chunks = n_chunks
    p.calls = calls
    p.chunk2call = chunk2call
    p.win_chunks = win_chunks
    p.call_first_window = call_first_window
    p.gid2ipos = gid2ipos
    p.idx_wrapped = [np.concatenate(
        [_wrap_idx(idx_data[c][o:o + n]) for (o, n, _q) in calls], axis=1)
        for c in range(N_CORES)]
    dl = np.where(dst_data < 0, SENT, dst_data)
    p.dstloc = []
    for c in range(N_CORES):
        byg = dl[c].reshape(n_chunks, 128).T
        byi = np.empty_like(byg)
        byi[:, gid2ipos] = byg
        p.dstloc.append(np.ascontiguousarray(byi.astype(BF16)))
    return p


# ---------------------------------------------------------------- builder


def build_nc(plans, n_local):
    QROWS = _qrows(n_local)
    WPQ = QROWS // WIN
    SHARD_ROWS = N_CORES * QROWS
    assert SHARD_ROWS < 32768
    P_NODES = QUARTERS * QROWS          # 12800
    HALF = P_NODES // 2                 # 6400
    PK = HALF // 128                    # windows per partition-half (50)
    n_win_real = (n_local + 127) // 128  # 98
    fdt = mybir.dt.float32
    bdt = mybir.dt.bfloat16
    iw_cols = [p.idx_wrapped[0].shape[1] for p in plans]
    dl_cols = [p.dstloc[0].shape[1] for p in plans]

    nc = bass.Bass(num_devices=N_CORES, num_swdge_queues=N_SWDGE_QUEUES)

    in_featT = nc.dram_tensor("in_featT", [IN_F, P_NODES], fdt, kind="ExternalInput")
    wts = {}
    for nm, shape, dt in [
            ("W1T", [IN_F, H], fdt), ("W2T", [128, H], fdt),
            ("M0T", [128, H], fdt), ("M1T", [128, H], fdt), ("M2T", [128, H], fdt),
            ("W4T", [128, C_OUT], fdt),
            ("b1c", [128, 1], fdt), ("b2c", [128, 1], fdt),
            ("b3c", [128, 1], fdt), ("b4c", [128, 1], fdt),
            ("iota", [128, 128], bdt), ("identT", [128, H], bdt)]:
        wts[nm] = nc.dram_tensor(nm, shape, dt, kind="ExternalInput")
    dinv_d = [nc.dram_tensor(f"dinv{r}", [128, HALF], fdt, kind="ExternalInput")
              for r in range(3)]
    idx_d = [nc.dram_tensor(f"idx{r}", [128, iw_cols[r]], mybir.dt.int16,
                            kind="ExternalInput") for r in range(3)]
    dstloc_d = [nc.dram_tensor(f"dstloc{r}", [128, dl_cols[r]], bdt,
                               kind="ExternalInput") for r in range(3)]
    out_d = nc.dram_tensor("out", [C_OUT, P_NODES], fdt, kind="ExternalOutput")

    # per propagation round t (0..5), per quarter q: bounce + shared table
    bounces = [[nc.dram_tensor(f"bounce{t}_{q}", [QROWS, 128], bdt)
                for q in range(QUARTERS)] for t in range(6)]
    tables = [[nc.dram_tensor(f"table{t}_{q}", [SHARD_ROWS, 128], bdt,
                              addr_space="Shared")
               for q in range(QUARTERS)] for t in range(6)]
    rg = [list(range(N_CORES))]

    Ident = mybir.ActivationFunctionType.Identity
    Cp = mybir.ActivationFunctionType.Copy

    def wpart(w):
        return 64 * (w // PK), (w % PK) * 128  # (partition base, column base)

    from contextlib import ExitStack
    with tile.TileContext(nc) as tc, ExitStack() as ctx:
        consts = ctx.enter_context(tc.tile_pool(name="consts", bufs=1))
        resid = ctx.enter_context(tc.tile_pool(name="resid", bufs=1))
        hpool = ctx.enter_context(tc.tile_pool(name="hpool", bufs=2))
        f1pool = ctx.enter_context(tc.tile_pool(name="f1pool", bufs=1))
        dinvp = ctx.enter_context(tc.tile_pool(name="dinvp", bufs=2))
        msgp = ctx.enter_context(tc.tile_pool(name="msgp", bufs=10))
        selp = ctx.enter_context(tc.tile_pool(name="selp", bufs=2))
        wtile = ctx.enter_context(tc.tile_pool(name="wtile", bufs=2))
        trp = ctx.enter_context(tc.tile_pool(name="trp", bufs=3))
        psum_seg = ctx.enter_context(
            tc.tile_pool(name="psum_seg", bufs=4, space="PSUM"))
        psum_aux = ctx.enter_context(
            tc.tile_pool(name="psum_aux", bufs=2, space="PSUM"))
        psum_tr = ctx.enter_context(
            tc.tile_pool(name="psum_tr", bufs=2, space="PSUM"))
        idxp = ctx.enter_context(tc.tile_pool(name="idxp", bufs=2))
        idxcp = ctx.enter_context(tc.tile_pool(name="idxcp", bufs=8))

        cw = {}
        for nm in wts:
            dt = bdt if nm in ("iota", "identT") else fdt
            cw[nm] = consts.tile(list(wts[nm].shape), dt, tag=nm, name=f"cw_{nm}")
            nc.sync.dma_start(out=cw[nm][:], in_=wts[nm][:])
        iota_t = cw["iota"]

        h_all = resid.tile([128, HALF], fdt)
        nc.vector.memset(h_all[:], 0.0)

        # ---- per-window table build + quarter AllGather trigger
        def build_window_table(w, src_tile, dinv_t, tbl_round):
            pb, cb = wpart(w)
            wn = min(WIN, n_local - w * WIN)
            q = w // WPQ
            scl = wtile.tile([128, WIN], bdt, tag="tblscl", name=f"scl_{tbl_round}_{w}")
            nc.vector.tensor_tensor(out=scl[pb:pb + 64, :wn],
                                    in0=src_tile[pb:pb + 64, cb:cb + wn],
                                    in1=dinv_t[pb:pb + 64, cb:cb + wn],
                                    op=mybir.AluOpType.mult)
            pt = psum_tr.tile([128, H], bdt, tag="tr", name=f"pt_{tbl_round}_{w}")
            nc.tensor.transpose(out=pt[:wn, :], in_=scl[pb:pb + 64, :wn],
                                identity=cw["identT"][pb:pb + 64, :])
            st = trp.tile([128, H], bdt, tag="trs", name=f"trs_{tbl_round}_{w}")
            nc.scalar.activation(st[:wn, :], pt[:wn, :], Cp)
            wq = w - q * WPQ
            nc.sync.dma_start(
                out=bounces[tbl_round][q][wq * WIN:wq * WIN + wn, 0:H],
                in_=st[:wn, :])
            # Trigger each quarter's AllGather a few windows past the quarter
            # boundary: the wait on the quarter's bounce DMAs is then on
            # already-drained writes, so the gpsimd gather stream never stalls
            # behind the trigger. (Collectives may only trigger from Pool/DMA
            # engines on trn2.)
            for q2 in range(QUARTERS):
                tw = min((q2 + 1) * WPQ, n_win_real) - 1
                if q2 < QUARTERS - 1:
                    tw = min(tw + 3, n_win_real - 1)
                if w == tw:
                    nc.gpsimd.collective_compute(
                        "AllGather", mybir.AluOpType.bypass, replica_groups=rg,
                        ins=[bounces[tbl_round][q2][:].opt()],
                        outs=[tables[tbl_round][q2][:].opt()])

        # ---- initial 2-layer MLP -> h (packed), builds table round 0
        dinv_t = dinvp.tile([128, HALF], fdt, tag="dinv")
        nc.sync.dma_start(out=dinv_t[:], in_=dinv_d[0][:])
        h_cur = hpool.tile([128, HALF], fdt, tag="h")
        for w in range(n_win_real):
            pb, cb = wpart(w)
            wn = min(WIN, n_local - w * WIN)
            infw = wtile.tile([IN_F, WIN], fdt, tag="infw", name=f"infw_{w}")
            nc.sync.dma_start(out=infw[:, :wn],
                              in_=in_featT[:, w * WIN:w * WIN + wn])
            ps = psum_aux.tile([128, WIN], fdt, tag="aux", name=f"mlpa_{w}")
            nc.tensor.matmul(out=ps[pb:pb + 64, :wn], lhsT=cw["W1T"][:],
                             rhs=infw[:, :wn], start=True, stop=True,
                             tile_position=(0, pb))
            y1 = wtile.tile([128, WIN], fdt, tag="y1", name=f"y1_{w}")
            nc.scalar.activation(y1[pb:pb + 64, :wn], ps[pb:pb + 64, :wn], Ident,
                                 bias=cw["b1c"][pb:pb + 64, :])
            h0w = wtile.tile([128, WIN], fdt, tag="h0w", name=f"h0w_{w}")
            nc.vector.scalar_tensor_tensor(
                out=h0w[pb:pb + 64, :wn], in0=y1[pb:pb + 64, :wn], scalar=0.01,
                in1=y1[pb:pb + 64, :wn],
                op0=mybir.AluOpType.mult, op1=mybir.AluOpType.max)
            ps2 = psum_aux.tile([128, WIN], fdt, tag="aux", name=f"mlpb_{w}")
            nc.tensor.matmul(out=ps2[pb:pb + 64, :wn],
                             lhsT=cw["W2T"][pb:pb + 64, :],
                             rhs=h0w[pb:pb + 64, :wn], start=True, stop=True,
                             tile_position=(pb, pb))
            y2 = wtile.tile([128, WIN], fdt, tag="y1", name=f"y2_{w}")
            nc.scalar.activation(y2[pb:pb + 64, :wn], ps2[pb:pb + 64, :wn], Ident,
                                 bias=cw["b2c"][pb:pb + 64, :])
            nc.vector.scalar_tensor_tensor(
                out=h_cur[pb:pb + 64, cb:cb + wn], in0=y2[pb:pb + 64, :wn],
                scalar=0.01, in1=y2[pb:pb + 64, :wn],
                op0=mybir.AluOpType.mult, op1=mybir.AluOpType.max)
            build_window_table(w, h_cur, dinv_t, 0)

        nreg_cache = {}

        def propagate(plan, idx_dram, dl_t, tbl_round, epilogue):
            batch = 16
            built = {}
            msg_tiles = {}
            emitted = [0]
            call_order = sorted(range(len(plan.calls)),
                                key=lambda c: (plan.call_first_window[c], c))
            iw_off = {}
            o = 0
            for cid, (_so, n, _q) in enumerate(plan.calls):
                iw_off[cid] = o
                o += n // 16

            def emit_calls(up_to_w):
                while emitted[0] < len(call_order):
                    cid = call_order[emitted[0]]
                    if plan.call_first_window[cid] > up_to_w:
                        break
                    so, n, q = plan.calls[cid]
                    nch = n // 128
                    mt = msgp.tile([128, nch, H], bdt, tag="msg", name=f"msg_{cid}")
                    it = idxcp.tile([128, CALL_MAX // 16], mybir.dt.int16,
                                    tag="idxc", name=f"idxc_{cid}")
                    nc.sync.dma_start(
                        out=it[:, 0:n // 16],
                        in_=idx_dram[:, iw_off[cid]:iw_off[cid] + n // 16])
                    if n not in nreg_cache:
                        nreg_cache[n] = nc.gpsimd.to_reg(n)
                    _dma_gather(
                        nc.gpsimd, out_ap=mt[:],
                        in_ap=tables[tbl_round][q][0:SHARD_ROWS, 0:H],
                        idxs_ap=it[:, 0:n // 16],
                        num_idxs=n, num_idxs_reg=nreg_cache[n],
                        elem_size=H, elem_step=128, queue_num=q % N_SWDGE_QUEUES)
                    msg_tiles[cid] = mt
                    emitted[0] += 1

            for w in range(plan.n_win):
                emit_calls(w)
                pb, _cb = wpart(w)
                wn = min(WIN, n_local - w * WIN)
                ps = psum_seg.tile([128, 512], fdt, tag="seg", name=f"seg_{w}")
                gids = plan.win_chunks[w]
                if not gids:
                    nc.vector.memset(ps[pb:pb + 64, :wn], 0.0)
                for k, gid in enumerate(gids):
                    ip = int(plan.gid2ipos[gid])
                    bi = ip // batch
                    if bi not in built:
                        i0 = bi * batch
                        nbi = min(batch, plan.n_chunks - i0)
                        st = selp.tile([128, batch * 128], bdt, tag="sel",
                                       name=f"sel_{bi}")
                        nc.vector.tensor_tensor(
                            out=st[:, 0:nbi * 128],
                            in0=dl_t[:, i0:i0 + nbi, None].to_broadcast(
                                [128, nbi, 128]),
                            in1=iota_t[:, None, :].to_broadcast([128, nbi, 128]),
                            op=mybir.AluOpType.is_equal)
                        built[bi] = st
                    st = built[bi]
                    cid, slot = plan.chunk2call[gid]
                    nc.tensor.matmul(
                        out=ps[pb:pb + 64, 0:WIN],
                        lhsT=msg_tiles[cid][:, slot, :],
                        rhs=st[:, (ip - bi * batch) * 128:
                               (ip - bi * batch) * 128 + 128],
                        start=(k == 0), stop=(k == len(gids) - 1),
                        tile_position=(0, pb), skip_group_check=True)
                epilogue(w, ps, wn)

        for r in range(3):
            dl_t = idxp.tile([128, dl_cols[r]], bdt, tag="dl", name=f"dl_{r}")
            nc.sync.dma_start(out=dl_t[:], in_=dstloc_d[r][:])

            f1 = f1pool.tile([128, HALF], fdt, tag="f1", name=f"f1_{r}")

            # round A (table 2r): produce f1, build table 2r+1 from f1*dinv
            def epi1(w, ps, wn, f1=f1, dinv_t=dinv_t, h_cur=h_cur, r=r):
                pb, cb = wpart(w)
                tmp = wtile.tile([128, WIN], fdt, tag="scaled", name=f"ta_{r}_{w}")
                nc.vector.tensor_tensor(out=tmp[pb:pb + 64, :wn],
                                        in0=ps[pb:pb + 64, :wn],
                                        in1=dinv_t[pb:pb + 64, cb:cb + wn],
                                        op=mybir.AluOpType.mult)
                nc.vector.tensor_tensor(out=f1[pb:pb + 64, cb:cb + wn],
                                        in0=h_cur[pb:pb + 64, cb:cb + wn],
                                        in1=tmp[pb:pb + 64, :wn],
                                        op=mybir.AluOpType.subtract)
                build_window_table(w, f1, dinv_t, 2 * r + 1)

            propagate(plans[r], idx_d[r], dl_t, 2 * r, epi1)

            h_new = hpool.tile([128, HALF], fdt, tag="h", name=f"hn_{r}")
            if r < 2:
                dinv_next = dinvp.tile([128, HALF], fdt, tag="dinv",
                                       name=f"dinv_{r + 1}")
                nc.sync.dma_start(out=dinv_next[:], in_=dinv_d[r + 1][:])
            else:
                dinv_next = None

            # round B (table 2r+1): produce h_new (+ h_all), build next
            # relation's table 2r+2 from h_new*dinv_{r+1}
            def epi2(w, ps, wn, f1=f1, dinv_t=dinv_t, h_cur=h_cur, h_new=h_new,
                     dinv_next=dinv_next, r=r):
                pb, cb = wpart(w)
                tmp = wtile.tile([128, WIN], fdt, tag="scaled", name=f"tb_{r}_{w}")
                nc.vector.tensor_tensor(out=tmp[pb:pb + 64, :wn],
                                        in0=ps[pb:pb + 64, :wn],
                                        in1=dinv_t[pb:pb + 64, cb:cb + wn],
                                        op=mybir.AluOpType.mult)
                f2w = wtile.tile([128, WIN], fdt, tag="f2w", name=f"f2_{r}_{w}")
                nc.vector.tensor_tensor(out=f2w[pb:pb + 64, :wn],
                                        in0=f1[pb:pb + 64, cb:cb + wn],
                                        in1=tmp[pb:pb + 64, :wn],
                                        op=mybir.AluOpType.subtract)
                ps3 = psum_aux.tile([128, WIN], fdt, tag="aux", name=f"w3_{r}_{w}")
                nc.tensor.matmul(out=ps3[pb:pb + 64, :wn],
                                 lhsT=cw["M0T"][pb:pb + 64, :],
                                 rhs=h_cur[pb:pb + 64, cb:cb + wn],
                                 start=True, stop=False, tile_position=(pb, pb))
                nc.tensor.matmul(out=ps3[pb:pb + 64, :wn],
                                 lhsT=cw["M1T"][pb:pb + 64, :],
                                 rhs=f1[pb:pb + 64, cb:cb + wn],
                                 start=False, stop=False, tile_position=(pb, pb))
                nc.tensor.matmul(out=ps3[pb:pb + 64, :wn],
                                 lhsT=cw["M2T"][pb:pb + 64, :],
                                 rhs=f2w[pb:pb + 64, :wn],
                                 start=False, stop=True, tile_position=(pb, pb))
                nc.scalar.activation(h_new[pb:pb + 64, cb:cb + wn],
                                     ps3[pb:pb + 64, :wn], Ident,
                                     bias=cw["b3c"][pb:pb + 64, :])
                nc.vector.tensor_tensor(out=h_all[pb:pb + 64, cb:cb + wn],
                                        in0=h_all[pb:pb + 64, cb:cb + wn],
                                        in1=h_new[pb:pb + 64, cb:cb + wn],
                                        op=mybir.AluOpType.add)
                if r < 2:
                    build_window_table(w, h_new, dinv_next, 2 * r + 2)

            propagate(plans[r], idx_d[r], dl_t, 2 * r + 1, epi2)
            h_cur = h_new
            if r < 2:
                dinv_t = dinv_next

        # ---- final head
        for w in range(n_win_real):
            pb, cb = wpart(w)
            wn = min(WIN, n_local - w * WIN)
            lw = wtile.tile([128, WIN], fdt, tag="lrelu", name=f"lr_{w}")
            nc.vector.scalar_tensor_tensor(
                out=lw[pb:pb + 64, :wn], in0=h_all[pb:pb + 64, cb:cb + wn],
                scalar=0.01, in1=h_all[pb:pb + 64, cb:cb + wn],
                op0=mybir.AluOpType.mult, op1=mybir.AluOpType.max)
            ps = psum_aux.tile([128, WIN], fdt, tag="aux", name=f"hd_{w}")
            nc.tensor.matmul(out=ps[pb:pb + C_OUT, :wn],
                             lhsT=cw["W4T"][pb:pb + 64, :],
                             rhs=lw[pb:pb + 64, :wn],
                             start=True, stop=True, tile_position=(pb, pb))
            ow = trp.tile([128, WIN], fdt, tag="ow", name=f"ow_{w}")
            nc.scalar.activation(ow[pb:pb + C_OUT, :wn], ps[pb:pb + C_OUT, :wn],
                                 Ident, bias=cw["b4c"][pb:pb + C_OUT, :])
            nc.sync.dma_start(out=out_d[0:C_OUT, w * WIN:w * WIN + wn],
                              in_=ow[pb:pb + C_OUT, :wn])

    _insert_library_loads(nc)
    _fix_sync_waits(nc)
    return nc


# ---------------------------------------------------------------- kernel


def prepare(inputs):
    in_feat = np.asarray(inputs["in_feat"], np.float32)
    N = in_feat.shape[0]
    n_local = N // N_CORES
    P_NODES = QUARTERS * _qrows(n_local)
    HALF = P_NODES // 2
    W1 = np.asarray(inputs["W1"], np.float32)
    b1 = np.asarray(inputs["b1"], np.float32)
    W2 = np.asarray(inputs["W2"], np.float32)
    b2 = np.asarray(inputs["b2"], np.float32)
    W3 = np.asarray(inputs["W3"], np.float32)
    b3 = np.asarray(inputs["b3"], np.float32)
    W4 = np.asarray(inputs["W4"], np.float32)
    b4 = np.asarray(inputs["b4"], np.float32)
    srcs = [np.asarray(inputs[f"src{r}"]).astype(np.int64) for r in range(3)]
    dsts = [np.asarray(inputs[f"dst{r}"]).astype(np.int64) for r in range(3)]

    W3a, W3b, W3c = W3[:, 0:H], W3[:, H:2 * H], W3[:, 2 * H:3 * H]
    M = [THETAS[0, k] * W3a + THETAS[1, k] * W3b + THETAS[2, k] * W3c
         for k in range(3)]

    dinvs = []
    for r in range(3):
        deg = np.bincount(dsts[r], minlength=N).astype(np.float32)
        dinvs.append(np.maximum(deg, 1.0) ** -0.5)

    plans = [_plan_relation(srcs[r], dsts[r], N, n_local) for r in range(3)]
    nc = build_nc(plans, n_local)

    def dup(a):
        return np.ascontiguousarray(np.concatenate([a, a], axis=0))

    def dupcol(b):
        col = np.zeros((128, 1), np.float32)
        col[0:len(b), 0] = b
        col[64:64 + len(b), 0] = b
        return col

    iota = np.tile(np.arange(128, dtype=np.float32)[None, :], (128, 1)).astype(BF16)
    identT = dup(np.eye(H, dtype=np.float32)).astype(BF16)
    in_featT = in_feat.T.copy()

    def pack(a):
        if a.ndim == 1:
            a = np.tile(a[None, :], (H, 1))
        return np.ascontiguousarray(
            np.concatenate([a[:, :HALF], a[:, HALF:]], axis=0))

    in_maps = []
    for c in range(N_CORES):
        m = {
            "in_featT": np.ascontiguousarray(
                np.pad(in_featT[:, c * n_local:(c + 1) * n_local],
                       ((0, 0), (0, P_NODES - n_local)))),
            "W1T": W1.T.copy(), "W2T": dup(W2.T), "M0T": dup(M[0].T),
            "M1T": dup(M[1].T), "M2T": dup(M[2].T), "W4T": dup(W4.T),
            "b1c": dupcol(b1), "b2c": dupcol(b2), "b3c": dupcol(b3),
            "b4c": dupcol(b4),
            "iota": iota, "identT": identT,
        }
        for r in range(3):
            dl = np.pad(dinvs[r][c * n_local:(c + 1) * n_local],
                        (0, P_NODES - n_local))
            m[f"dinv{r}"] = pack(dl)
            m[f"idx{r}"] = np.ascontiguousarray(plans[r].idx_wrapped[c])
            m[f"dstloc{r}"] = np.ascontiguousarray(plans[r].dstloc[c])
        in_maps.append(m)
    return nc, in_maps, n_local


def kernel(**inputs):
    global LAST_BUILD
    nc, in_maps, n_local = prepare(inputs)
    _lower_library_reloads(nc)
    LAST_BUILD = (nc, in_maps)
    from concourse.bass_utils import run_bass_kernel_spmd
    res = run_bass_kernel_spmd(nc, in_maps, core_ids=list(range(N_CORES)))
    outs = [res.results[c]["out"][:, :n_local] for c in range(N_CORES)]
    return np.ascontiguousarray(np.concatenate(outs, axis=1).T)



# revision 54
# speedup vs baseline: 4.5288x; 4.5288x over previous
"""BWGNN-Hetero forward on 8 Trainium2 NeuronCores.

Node-sharded (N/8 nodes per core). Per relation: two polynomial-propagation
steps; segment-sum gathers per-edge source rows (dma_gather, bf16 tables
with 256B row stride) and reduces them with PE matmuls against
on-device-built one-hot selection matrices into 128-node PSUM windows.

The node table is split into 4 QUARTER tables (one per SWDGE queue): local
rows [3200q, 3200(q+1)) of every core are AllGathered into table_q
[8*3200=25600, 128] (int16-indexable). Gather calls for quarter q run on
SWDGE queue q, so descriptor generation is never ring-stalled behind a
single queue and the 4 rings drain concurrently. Each quarter's AllGather
is triggered as soon as the epilogue finishes that quarter's 25 windows,
overlapping the collective with the remaining descriptor generation.

Node state is feature-major and HALF-PACKED: a [128, P/2] tile holds
features of nodes [0,P/2) on partitions 0:64 and of [P/2,P) on partitions
64:128 (matmuls address the upper half via tile_position).

SPMD: one program for all 8 cores; the edge layout is padded to a common
structure (per-(window,quarter) chunk capacity = max over cores) so the
instruction stream is core-invariant while indices/dst data are inputs.
"""

import os
import numpy as np
import ml_dtypes

import concourse.bass as bass
import concourse.mybir as mybir
import concourse.tile as tile
from concourse import ap_utils
from concourse.bass import MemorySpace

N_CORES = 8
H = 64
C_OUT = 2
IN_F = 128
WIN = 128
CALL_MAX = int(os.environ.get("K_CALLMAX", "1024"))
QUARTERS = 2          # gather tables (== packed partition halves)
NCLS = 2 * QUARTERS   # gather classes: (table, src-node parity)
SPLIT_W = int(os.environ.get("K_SPLITW", "49"))  # windows in AG group 0 (49 = one AG per quarter)
SENT = 1024.0
N_SWDGE_QUEUES = int(os.environ.get("K_QUEUES", "4"))
SKIP_COLL = os.environ.get("K_SKIP_COLL", "0") == "1"
SKIP_GATHER = os.environ.get("K_SKIP_GATHER", "0") == "1"
SKIP_MM = os.environ.get("K_SKIP_MM", "0") == "1"
TINY = os.environ.get("K_TINY", "0") == "1"
TINY_MIN = os.environ.get("K_TINY_MIN", "0") == "1"
MSGP_BUFS = int(os.environ.get("K_MSGP_BUFS", "10"))
SCRATCH = int(os.environ.get("K_SCRATCH", "16384"))


def _qrows(n_local):
    """Padded local rows per quarter (window-aligned)."""
    return -(-n_local // (QUARTERS * WIN)) * WIN

THETAS = np.array([[3.0, -3.0, 0.75],
                   [0.0, 3.0, -1.50],
                   [0.0, 0.0, 0.75]], dtype=np.float32)

BF16 = ml_dtypes.bfloat16
LAST_BUILD = None

# ---------------------------------------------------------------- bir fixes


def _fix_sync_waits(nc, max_waits=1):
    """This walrus build rejects >1 sync-wait per instruction; move excess
    waits onto fresh nops on the same engine queue."""
    counter = [0]
    for fn in nc.m.functions:
        for bb in fn.blocks:
            new_insts = []
            for inst in bb.instructions:
                si = inst.sync_info
                if si is None or not si.on_wait or len(si.on_wait) <= max_waits:
                    new_insts.append(inst)
                    continue
                waits = list(si.on_wait)
                for w in waits[max_waits:]:
                    counter[0] += 1
                    nop = mybir.InstNoOp(name=f"waitsplit_{counter[0]}", ins=[], outs=[])
                    nop.engine = inst.engine
                    nop.sync_info = mybir.SyncInfo(on_wait=[w], on_update=[])
                    nc.register_instruction(nop)
                    new_insts.append(nop)
                inst.sync_info = mybir.SyncInfo(
                    on_wait=waits[:max_waits], on_update=list(si.on_update))
                new_insts.append(inst)
            if len(new_insts) != len(bb.instructions):
                bb.instructions[:] = new_insts


def _insert_library_loads(nc):
    import bass_rust as _bass_rust
    from concourse.library_config import all_libraries, standard
    mask = {}
    for lib in all_libraries:
        for t in lib.instructions:
            mask[t] = mask.get(t, 0) | (1 << lib.index)
    _bass_rust.insert_library_loads(nc, mask, len(all_libraries), standard.index)


def _lower_library_reloads(nc):
    """Rewrite the pseudo library-reload into the raw 64B PSEUDO_INST struct
    this walrus can encode (not sim-executable; call only before HW runs)."""
    import bass_rust as _bass_rust
    isa = nc.isa
    PO = isa.get_enum("NEURON_ISA_TPB_PSEUDO_OPCODE")
    for fn in nc.m.functions:
        for bb in fn.blocks:
            for i, inst in enumerate(bb.instructions):
                if not isinstance(inst, _bass_rust.InstPseudoReloadLibraryIndex):
                    continue
                raw = nc.engines[inst.engine]._isa(
                    isa.Opcode.NEURON_ISA_TPB_OPCODE_PSEUDO_INST,
                    {"pseudo_opcode":
                         PO.NEURON_ISA_TPB_PSEUDO_OPCODE_PSEUDO_LIBRARY_RELOAD_INDEX.value,
                     "lib_index": inst.lib_index},
                    "NEURON_ISA_TPB_PSEUDO_LIBRARY_RELOAD_INDEX_STRUCT",
                    [], [], True)
                raw.engine = inst.engine
                raw.sync_info = inst.sync_info
                nc.register_instruction(raw, overwrite=True)
                bb.instructions[i] = raw


def _dma_gather(gp, out_ap, in_ap, idxs_ap, num_idxs, num_idxs_reg, elem_size,
                elem_step, queue_num=0):
    """dma_gather with the elem_size%256 assert relaxed (row stride must
    still be a 256B multiple; validated on HW)."""
    assert idxs_ap.dtype == mybir.dt.int16
    assert in_ap.dtype == out_ap.dtype
    assert in_ap.space == MemorySpace.DRAM
    assert idxs_ap.space == MemorySpace.SBUF and out_ap.space == MemorySpace.SBUF
    assert ap_utils.ap_is_contiguous(out_ap.ap[1:])
    assert ap_utils.ap_is_contiguous(idxs_ap.ap[1:])
    assert in_ap.ap[-1][1] == out_ap.ap[-1][1] == elem_size
    assert out_ap.ap[0][1] * out_ap.ap[1][1] == ((num_idxs + 127) // 128) * 128
    assert in_ap.ap[0][0] == elem_step
    stride_bytes = elem_step * mybir.dt.size(in_ap.dtype)
    assert stride_bytes % 256 == 0 and stride_bytes // 256 < 256
    _in_ap = gp.lower_ap_dma(in_ap, for_custom_bir_dma=True)
    _idxs_ap = gp.lower_ap(idxs_ap)
    _out_ap = gp.lower_ap(out_ap)
    return gp.add_instruction(
        mybir.InstDMAGatherAnt(
            name=gp.bass.get_next_instruction_name(),
            ins=[*_in_ap, _idxs_ap, gp.lower_val_access(gp.to_reg(num_idxs_reg))],
            outs=[_out_ap],
            transpose=False, num_idxs=num_idxs, elem_size=elem_size,
            stride_bytes_256=stride_bytes // 256, gen_mode=0,
            single_packet=True, queue_num=queue_num, sbuf_tokens_per_rank=0,
            sbuf_free_dim_per_rank=0, sbuf_free_dim_pad_per_rank=0,
            sbuf_byte_offset=0))


# ---------------------------------------------------------------- host plan


def _wrap_idx(idx):
    """[n] -> [128, n/16] int16: idx i at [i%16, i//16], replicated for the
    8 GPSIMD cores across partition groups of 16."""
    n = len(idx)
    assert n % 16 == 0
    w = np.ascontiguousarray(idx.astype(np.int16).reshape(n // 16, 16).T)
    return np.tile(w, (8, 1))


class RelPlan:
    """Common (cross-core) structure + per-core data for one relation."""


def _plan_relation(src, dst, N, n_local):
    n_win = (n_local + WIN - 1) // WIN
    QROWS = _qrows(n_local)

    cores = []
    counts = np.zeros((N_CORES, n_win, NCLS), np.int64)
    for c in range(N_CORES):
        lo = c * n_local
        m = (dst >= lo) & (dst < lo + n_local)
        s = src[m]
        d = dst[m] - lo
        cs = s // n_local
        li = s % n_local
        lq = li % QROWS
        # class = (gather table, src parity); row = pair-row in the table.
        # Each quarter-table is AllGathered in two window-range groups that
        # land in disjoint row ranges: g0 = first SPLIT_W windows (rows
        # [0, 8*G0P)), g1 = the rest (rows [8*G0P, ...)), core-major inside.
        q = (li // QROWS) * 2 + (lq % 2)
        g0n = SPLIT_W * WIN
        g0p = g0n // 2
        g1p = (QROWS - g0n) // 2
        g = (lq >= g0n).astype(np.int64)
        lg = lq - g * g0n
        row = g * (N_CORES * g0p) + cs * np.where(g == 0, g0p, g1p) + lg // 2
        w = d // WIN
        order = np.lexsort((d, w, q))
        row, d, q, w = row[order], d[order], q[order], w[order]
        counts[c] = np.bincount(w * NCLS + q,
                                minlength=n_win * NCLS).reshape(n_win, NCLS)
        cores.append((row, d, q, w))

    cap = np.ceil(counts.max(axis=0) / 128).astype(np.int64)

    chunk_ws = []
    chunk_base = np.zeros((n_win, NCLS), np.int64)
    quarter_first_chunk = []
    g = 0
    for q in range(NCLS):
        quarter_first_chunk.append(g)
        for w in range(n_win):
            chunk_base[w, q] = g
            for _ in range(cap[w, q]):
                chunk_ws.append((w, q))
                g += 1
    n_chunks = g
    n_slots = n_chunks * 128

    # window-aligned calls: a call covers whole windows of one class, so a
    # window's matmuls never straddle two calls of the same class (keeps the
    # msg-pool wait chain linear -> no tile-pool deadlock).
    calls = []
    chunk2call = {}
    max_chunks = CALL_MAX // 128
    for q in range(NCLS):
        w = 0
        while w < n_win:
            c0 = chunk_base[w, q]
            w2 = w
            take = 0
            while w2 < n_win and take + cap[w2, q] <= max_chunks:
                take += cap[w2, q]
                w2 += 1
            assert take > 0
            cid = len(calls)
            calls.append((int(c0) * 128, int(take) * 128, q))
            for j in range(take):
                chunk2call[int(c0) + j] = (cid, j)
            w = w2

    win_chunks = [[] for _ in range(n_win)]
    for gid, (w, q) in enumerate(chunk_ws):
        win_chunks[w].append(gid)
    gid2ipos = np.zeros(max(1, n_chunks), np.int64)
    ipos = 0
    for w in range(n_win):
        for gid in win_chunks[w]:
            gid2ipos[gid] = ipos
            ipos += 1
    call_first_window = [min((chunk_ws[g][0] for g in
                              range(calls[cid][0] // 128,
                                    calls[cid][0] // 128 + calls[cid][1] // 128)),
                             default=0)
                         for cid in range(len(calls))]
    # idx columns are laid out in consumption order so per-call idx loads can
    # be batched into one DMA per group of consecutive calls.
    call_order = sorted(range(len(calls)),
                        key=lambda c: (call_first_window[c], c))
    iw_off = {}
    o = 0
    for cid in call_order:
        iw_off[cid] = o
        o += calls[cid][1] // 16

    idx_data = np.zeros((N_CORES, n_slots), np.int64)
    dst_data = np.full((N_CORES, n_slots), -1.0, np.float64)
    for c in range(N_CORES):
        row, d, q, w = cores[c]
        pos = 0
        for qv in range(NCLS):
            for wv in range(n_win):
                cnt = counts[c, wv, qv]
                if cnt == 0:
                    continue
                b = chunk_base[wv, qv] * 128
                idx_data[c, b:b + cnt] = row[pos:pos + cnt]
                dst_data[c, b:b + cnt] = d[pos:pos + cnt] - wv * WIN
                pos += cnt
        assert pos == len(row)

    p = RelPlan()
    p.n_win = n_win
    p.n_chunks = n_chunks
    p.calls = calls
    p.chunk2call = chunk2call
    p.win_chunks = win_chunks
    p.call_first_window = call_first_window
    p.call_order = call_order
    p.iw_off = iw_off
    p.gid2ipos = gid2ipos
    p.idx_wrapped = [np.concatenate(
        [_wrap_idx(idx_data[c][calls[cid][0]:calls[cid][0] + calls[cid][1]])
         for cid in call_order], axis=1)
        for c in range(N_CORES)]
    dl = np.where(dst_data < 0, -1.0, dst_data)
    p.dstloc = []
    for c in range(N_CORES):
        byg = dl[c].reshape(n_chunks, 128).T
        byi = np.empty_like(byg)
        byi[:, gid2ipos] = byg
        p.dstloc.append(np.ascontiguousarray(byi.astype(np.int8)))
    return p


# ---------------------------------------------------------------- builder


def build_nc(plans, n_local):
    QROWS = _qrows(n_local)
    WPQ = QROWS // WIN
    SHARD_ROWS = N_CORES * (QROWS // 2)   # pair-rows (2 nodes per 256B row)
    assert SHARD_ROWS < 32768
    P_NODES = QUARTERS * QROWS          # 12800
    HALF = P_NODES // 2                 # 6400
    PK = HALF // 128                    # windows per partition-half (50)
    n_win_real = (n_local + 127) // 128  # 98
    fdt = mybir.dt.float32
    bdt = mybir.dt.bfloat16
    iw_cols = [p.idx_wrapped[0].shape[1] for p in plans]
    dl_cols = [p.dstloc[0].shape[1] for p in plans]

    nc = bass.Bass(num_devices=N_CORES, num_swdge_queues=N_SWDGE_QUEUES,
                   dynamic_dma_scratch_size=SCRATCH)
    idt = mybir.dt.int8

    in_featT = (None if TINY_MIN else
                nc.dram_tensor("in_featT", [IN_F, P_NODES], bdt,
                               kind="ExternalInput"))
    wts = {}
    for nm, shape, dt in [
            ("W1T", [IN_F, H], bdt), ("W2T", [128, H], fdt),
            ("M0T", [128, H], fdt), ("M1T", [128, H], fdt), ("M2T", [128, H], fdt),
            ("W4T", [128, C_OUT], fdt),
            ("b1c", [128, 1], fdt), ("b2c", [128, 1], fdt),
            ("b3c", [128, 1], fdt), ("b4c", [128, 1], fdt),
            ("iota", [128, 128], idt), ("identT", [128, H], bdt)]:
        wts[nm] = nc.dram_tensor(nm, shape, dt, kind="ExternalInput")
    if TINY_MIN:
        dinv_d = idx_d = dstloc_d = [None] * 3
    else:
        dinv_d = [nc.dram_tensor(f"dinv{r}", [2, HALF], fdt,
                                 kind="ExternalInput") for r in range(3)]
        idx_d = [nc.dram_tensor(f"idx{r}", [128, iw_cols[r]], mybir.dt.int16,
                                kind="ExternalInput") for r in range(3)]
        dstloc_d = [nc.dram_tensor(f"dstloc{r}", [128, dl_cols[r]], idt,
                                   kind="ExternalInput") for r in range(3)]
    out_d = nc.dram_tensor("out", [C_OUT, P_NODES], fdt, kind="ExternalOutput")

    # per propagation round t (0..5), per table-half q: compact node-major
    # bounce [QROWS, 64] (= [QROWS/2, 128] pair-rows) + AllGathered table
    bounces = [[nc.dram_tensor(f"bounce{t}_{q}", [QROWS, H], bdt)
                for q in range(QUARTERS)] for t in range(6)]
    tables = [[nc.dram_tensor(f"table{t}_{q}", [SHARD_ROWS, 128], bdt,
                              addr_space="Shared")
               for q in range(QUARTERS)] for t in range(6)]
    rg = [list(range(N_CORES))]

    Ident = mybir.ActivationFunctionType.Identity
    Cp = mybir.ActivationFunctionType.Copy

    def wpart(w):
        return 64 * (w // PK), (w % PK) * 128  # (partition base, column base)

    from contextlib import ExitStack
    with tile.TileContext(nc) as tc, ExitStack() as ctx:
        consts = ctx.enter_context(tc.tile_pool(name="consts", bufs=1))
        resid = ctx.enter_context(tc.tile_pool(name="resid", bufs=1))
        hpool = ctx.enter_context(tc.tile_pool(name="hpool", bufs=2))
        f1pool = ctx.enter_context(tc.tile_pool(name="f1pool", bufs=1))
        dinvp = ctx.enter_context(tc.tile_pool(name="dinvp", bufs=2))
        msgp = ctx.enter_context(tc.tile_pool(name="msgp", bufs=MSGP_BUFS))
        selp = ctx.enter_context(tc.tile_pool(name="selp", bufs=2))
        wtile = ctx.enter_context(tc.tile_pool(name="wtile", bufs=2))
        trp = ctx.enter_context(tc.tile_pool(name="trp", bufs=3))
        psum_seg = ctx.enter_context(
            tc.tile_pool(name="psum_seg", bufs=4, space="PSUM"))
        psum_aux = ctx.enter_context(
            tc.tile_pool(name="psum_aux", bufs=2, space="PSUM"))
        psum_tr = ctx.enter_context(
            tc.tile_pool(name="psum_tr", bufs=2, space="PSUM"))
        idxp = ctx.enter_context(tc.tile_pool(name="idxp", bufs=2))
        idxcp = ctx.enter_context(tc.tile_pool(name="idxcp", bufs=8))

        cw = {}
        for nm in wts:
            dt = (idt if nm == "iota"
                  else bdt if nm in ("identT", "W1T") else fdt)
            cw[nm] = consts.tile(list(wts[nm].shape), dt, tag=nm, name=f"cw_{nm}")
            nc.sync.dma_start(out=cw[nm][:], in_=wts[nm][:])
        iota_t = cw["iota"]

        def load_dinv(dst_tile, r):
            for hh in range(2):
                nc.sync.dma_start(
                    out=dst_tile[hh * 64:(hh + 1) * 64, :],
                    in_=dinv_d[r][hh:hh + 1, :].to_broadcast([64, HALF]))

        h_all = resid.tile([128, HALF], fdt)
        nc.vector.memset(h_all[:], 0.0)

        # ---- per-window table build + quarter AllGather trigger
        def build_window_table(w, src_tile, dinv_t, tbl_round):
            pb, cb = wpart(w)
            wn = min(WIN, n_local - w * WIN)
            q = w // WPQ
            scl = wtile.tile([128, WIN], bdt, tag="tblscl", name=f"scl_{tbl_round}_{w}")
            nc.vector.tensor_tensor(out=scl[pb:pb + 64, :wn],
                                    in0=src_tile[pb:pb + 64, cb:cb + wn],
                                    in1=dinv_t[pb:pb + 64, cb:cb + wn],
                                    op=mybir.AluOpType.mult)
            pt = psum_tr.tile([128, H], bdt, tag="tr", name=f"pt_{tbl_round}_{w}")
            nc.tensor.transpose(out=pt[:wn, :], in_=scl[pb:pb + 64, :wn],
                                identity=cw["identT"][pb:pb + 64, :])
            st = trp.tile([128, H], bdt, tag="trs", name=f"trs_{tbl_round}_{w}")
            nc.scalar.activation(st[:wn, :], pt[:wn, :], Cp)
            wq = w - q * WPQ
            nc.scalar.dma_start(
                out=bounces[tbl_round][q][wq * WIN:wq * WIN + wn, 0:H],
                in_=st[:wn, :])
            # Trigger each (quarter, window-group) AllGather a few windows
            # past the group's last window: the wait on the group's bounce
            # DMAs is then on already-drained writes, so the gpsimd gather
            # stream never stalls behind the trigger. (Collectives may only
            # trigger from Pool/DMA engines on trn2.)  The two groups of a
            # quarter land in disjoint row ranges of the same table, keeping
            # the gather-class count (and edge padding) unchanged while the
            # last, blocking AllGather is only half a quarter.
            g0n = SPLIT_W * WIN
            g0p = g0n // 2
            for q2 in range(QUARTERS):
                for g2 in range(2):
                    if g2 == 1 and SPLIT_W == WPQ:
                        continue
                    lg2 = 1 if SPLIT_W < WPQ else 0
                    lw = q2 * WPQ + (SPLIT_W - 1 if g2 == 0 else WPQ - 1)
                    last = (q2 == QUARTERS - 1 and g2 == lg2)
                    tw = lw if last else min(lw + 3, n_win_real - 1)
                    if w == tw and not SKIP_COLL:
                        if g2 == 0:
                            b_ap = bounces[tbl_round][q2][0:g0n, :]
                            t_ap = tables[tbl_round][q2][0:N_CORES * g0p, :]
                        else:
                            b_ap = bounces[tbl_round][q2][g0n:QROWS, :]
                            t_ap = tables[tbl_round][q2][
                                N_CORES * g0p:SHARD_ROWS, :]
                        nc.gpsimd.collective_compute(
                            "AllGather", mybir.AluOpType.bypass,
                            replica_groups=rg,
                            ins=[b_ap.opt()], outs=[t_ap.opt()])

        # ---- initial 2-layer MLP -> h (packed), builds table round 0
        dinv_t = dinvp.tile([128, HALF], fdt, tag="dinv")
        if TINY_MIN:
            nc.vector.memset(dinv_t[:], 0.0)
        else:
            load_dinv(dinv_t, 0)
        h_cur = hpool.tile([128, HALF], fdt, tag="h")
        for w in range(0 if TINY else n_win_real):
            pb, cb = wpart(w)
            wn = min(WIN, n_local - w * WIN)
            infw = wtile.tile([IN_F, WIN], bdt, tag="infw", name=f"infw_{w}")
            nc.sync.dma_start(out=infw[:, :wn],
                              in_=in_featT[:, w * WIN:w * WIN + wn])
            ps = psum_aux.tile([128, WIN], fdt, tag="aux", name=f"mlpa_{w}")
            nc.tensor.matmul(out=ps[pb:pb + 64, :wn], lhsT=cw["W1T"][:],
                             rhs=infw[:, :wn], start=True, stop=True,
                             tile_position=(0, pb))
            y1 = wtile.tile([128, WIN], fdt, tag="y1", name=f"y1_{w}")
            nc.scalar.activation(y1[pb:pb + 64, :wn], ps[pb:pb + 64, :wn], Ident,
                                 bias=cw["b1c"][pb:pb + 64, :])
            h0w = wtile.tile([128, WIN], fdt, tag="h0w", name=f"h0w_{w}")
            nc.vector.scalar_tensor_tensor(
                out=h0w[pb:pb + 64, :wn], in0=y1[pb:pb + 64, :wn], scalar=0.01,
                in1=y1[pb:pb + 64, :wn],
                op0=mybir.AluOpType.mult, op1=mybir.AluOpType.max)
            ps2 = psum_aux.tile([128, WIN], fdt, tag="aux", name=f"mlpb_{w}")
            nc.tensor.matmul(out=ps2[pb:pb + 64, :wn],
                             lhsT=cw["W2T"][pb:pb + 64, :],
                             rhs=h0w[pb:pb + 64, :wn], start=True, stop=True,
                             tile_position=(pb, pb))
            y2 = wtile.tile([128, WIN], fdt, tag="y1", name=f"y2_{w}")
            nc.scalar.activation(y2[pb:pb + 64, :wn], ps2[pb:pb + 64, :wn], Ident,
                                 bias=cw["b2c"][pb:pb + 64, :])
            nc.vector.scalar_tensor_tensor(
                out=h_cur[pb:pb + 64, cb:cb + wn], in0=y2[pb:pb + 64, :wn],
                scalar=0.01, in1=y2[pb:pb + 64, :wn],
                op0=mybir.AluOpType.mult, op1=mybir.AluOpType.max)
            build_window_table(w, h_cur, dinv_t, 0)

        nreg_cache = {}

        def propagate(plan, idx_dram, dl_t, tbl_round, epilogue):
            batch = 16
            IB = 8           # gather calls per batched idx DMA
            built = {}
            msg_tiles = {}
            emitted = [0]
            call_order = plan.call_order
            iw_off = plan.iw_off
            ibatch = [None, 0, 0]    # tile, base col, end col

            def emit_calls(up_to_w):
                while emitted[0] < len(call_order):
                    pos = emitted[0]
                    cid = call_order[pos]
                    if plan.call_first_window[cid] > up_to_w:
                        break
                    so, n, q = plan.calls[cid]
                    nch = n // 128
                    if iw_off[cid] >= ibatch[2]:
                        c0 = iw_off[cid]
                        c1 = c0
                        for cid2 in call_order[pos:pos + IB]:
                            c1 = iw_off[cid2] + plan.calls[cid2][1] // 16
                        it = idxcp.tile([128, IB * (CALL_MAX // 16)],
                                        mybir.dt.int16, tag="idxc",
                                        name=f"idxc_{tbl_round}_{pos}")
                        nc.sync.dma_start(out=it[:, 0:c1 - c0],
                                          in_=idx_dram[:, c0:c1])
                        ibatch[0], ibatch[1], ibatch[2] = it, c0, c1
                    mt = msgp.tile([128, nch, H], bdt, tag="msg", name=f"msg_{cid}")
                    b0 = iw_off[cid] - ibatch[1]
                    if n not in nreg_cache:
                        nreg_cache[n] = nc.gpsimd.to_reg(n)
                    if not SKIP_GATHER:
                        _dma_gather(
                            nc.gpsimd, out_ap=mt[:],
                            in_ap=tables[tbl_round][q >> 1][
                                0:SHARD_ROWS, (q & 1) * H:(q & 1) * H + H],
                            idxs_ap=ibatch[0][:, b0:b0 + n // 16],
                            num_idxs=n, num_idxs_reg=nreg_cache[n],
                            elem_size=H, elem_step=128, queue_num=q % N_SWDGE_QUEUES)
                    else:
                        nc.vector.memset(mt[:], 0.0)
                    msg_tiles[cid] = mt
                    emitted[0] += 1

            for w in range(plan.n_win):
                emit_calls(w)
                pb, _cb = wpart(w)
                wn = min(WIN, n_local - w * WIN)
                ps = psum_seg.tile([128, 512], fdt, tag="seg", name=f"seg_{w}")
                gids = plan.win_chunks[w]
                if SKIP_MM:
                    gids = []
                if not gids:
                    nc.vector.memset(ps[pb:pb + 64, :wn], 0.0)
                for k, gid in enumerate(gids):
                    ip = int(plan.gid2ipos[gid])
                    bi = ip // batch
                    if bi not in built:
                        i0 = bi * batch
                        nbi = min(batch, plan.n_chunks - i0)
                        st = selp.tile([128, batch * 128], bdt, tag="sel",
                                       name=f"sel_{bi}")
                        nc.vector.tensor_tensor(
                            out=st[:, 0:nbi * 128],
                            in0=dl_t[:, i0:i0 + nbi, None].to_broadcast(
                                [128, nbi, 128]),
                            in1=iota_t[:, None, :].to_broadcast([128, nbi, 128]),
                            op=mybir.AluOpType.is_equal)
                        built[bi] = st
                    st = built[bi]
                    cid, slot = plan.chunk2call[gid]
                    nc.tensor.matmul(
                        out=ps[pb:pb + 64, 0:WIN],
                        lhsT=msg_tiles[cid][:, slot, :],
                        rhs=st[:, (ip - bi * batch) * 128:
                               (ip - bi * batch) * 128 + 128],
                        start=(k == 0), stop=(k == len(gids) - 1),
                        tile_position=(0, pb), skip_group_check=True)
                epilogue(w, ps, wn)

        for r in (() if TINY else range(3)):
            dl_t = idxp.tile([128, dl_cols[r]], idt, tag="dl", name=f"dl_{r}")
            nc.sync.dma_start(out=dl_t[:], in_=dstloc_d[r][:])

            f1 = f1pool.tile([128, HALF], fdt, tag="f1", name=f"f1_{r}")

            # round A (table 2r): produce f1, build table 2r+1 from f1*dinv
            def epi1(w, ps, wn, f1=f1, dinv_t=dinv_t, h_cur=h_cur, r=r):
                pb, cb = wpart(w)
                tmp = wtile.tile([128, WIN], fdt, tag="scaled", name=f"ta_{r}_{w}")
                nc.vector.tensor_tensor(out=tmp[pb:pb + 64, :wn],
                                        in0=ps[pb:pb + 64, :wn],
                                        in1=dinv_t[pb:pb + 64, cb:cb + wn],
                                        op=mybir.AluOpType.mult)
                nc.vector.tensor_tensor(out=f1[pb:pb + 64, cb:cb + wn],
                                        in0=h_cur[pb:pb + 64, cb:cb + wn],
                                        in1=tmp[pb:pb + 64, :wn],
                                        op=mybir.AluOpType.subtract)
                build_window_table(w, f1, dinv_t, 2 * r + 1)

            propagate(plans[r], idx_d[r], dl_t, 2 * r, epi1)

            h_new = hpool.tile([128, HALF], fdt, tag="h", name=f"hn_{r}")
            if r < 2:
                dinv_next = dinvp.tile([128, HALF], fdt, tag="dinv",
                                       name=f"dinv_{r + 1}")
                load_dinv(dinv_next, r + 1)
            else:
                dinv_next = None

            # round B (table 2r+1): produce h_new (+ h_all), build next
            # relation's table 2r+2 from h_new*dinv_{r+1}
            def epi2(w, ps, wn, f1=f1, dinv_t=dinv_t, h_cur=h_cur, h_new=h_new,
                     dinv_next=dinv_next, r=r):
                pb, cb = wpart(w)
                tmp = wtile.tile([128, WIN], fdt, tag="scaled", name=f"tb_{r}_{w}")
                nc.vector.tensor_tensor(out=tmp[pb:pb + 64, :wn],
                                        in0=ps[pb:pb + 64, :wn],
                                        in1=dinv_t[pb:pb + 64, cb:cb + wn],
                                        op=mybir.AluOpType.mult)
                f2w = wtile.tile([128, WIN], fdt, tag="f2w", name=f"f2_{r}_{w}")
                nc.vector.tensor_tensor(out=f2w[pb:pb + 64, :wn],
                                        in0=f1[pb:pb + 64, cb:cb + wn],
                                        in1=tmp[pb:pb + 64, :wn],
                                        op=mybir.AluOpType.subtract)
                ps3 = psum_aux.tile([128, WIN], fdt, tag="aux", name=f"w3_{r}_{w}")
                nc.tensor.matmul(out=ps3[pb:pb + 64, :wn],
                                 lhsT=cw["M0T"][pb:pb + 64, :],
                                 rhs=h_cur[pb:pb + 64, cb:cb + wn],
                                 start=True, stop=False, tile_position=(pb, pb))
                nc.tensor.matmul(out=ps3[pb:pb + 64, :wn],
                                 lhsT=cw["M1T"][pb:pb + 64, :],
                                 rhs=f1[pb:pb + 64, cb:cb + wn],
                                 start=False, stop=False, tile_position=(pb, pb))
                nc.tensor.matmul(out=ps3[pb:pb + 64, :wn],
                                 lhsT=cw["M2T"][pb:pb + 64, :],
                                 rhs=f2w[pb:pb + 64, :wn],
                                 start=False, stop=True, tile_position=(pb, pb))
                nc.scalar.activation(h_new[pb:pb + 64, cb:cb + wn],
                                     ps3[pb:pb + 64, :wn], Ident,
                                     bias=cw["b3c"][pb:pb + 64, :])
                nc.vector.tensor_tensor(out=h_all[pb:pb + 64, cb:cb + wn],
                                        in0=h_all[pb:pb + 64, cb:cb + wn],
                                        in1=h_new[pb:pb + 64, cb:cb + wn],
                                        op=mybir.AluOpType.add)
                if r < 2:
                    build_window_table(w, h_new, dinv_next, 2 * r + 2)

            propagate(plans[r], idx_d[r], dl_t, 2 * r + 1, epi2)
            h_cur = h_new
            if r < 2:
                dinv_t = dinv_next

        # ---- final head
        for w in range(n_win_real):
            pb, cb = wpart(w)
            wn = min(WIN, n_local - w * WIN)
            lw = wtile.tile([128, WIN], fdt, tag="lrelu", name=f"lr_{w}")
            nc.vector.scalar_tensor_tensor(
                out=lw[pb:pb + 64, :wn], in0=h_all[pb:pb + 64, cb:cb + wn],
                scalar=0.01, in1=h_all[pb:pb + 64, cb:cb + wn],
                op0=mybir.AluOpType.mult, op1=mybir.AluOpType.max)
            ps = psum_aux.tile([128, WIN], fdt, tag="aux", name=f"hd_{w}")
            nc.tensor.matmul(out=ps[pb:pb + C_OUT, :wn],
                             lhsT=cw["W4T"][pb:pb + 64, :],
                             rhs=lw[pb:pb + 64, :wn],
                             start=True, stop=True, tile_position=(pb, pb))
            ow = trp.tile([128, WIN], fdt, tag="ow", name=f"ow_{w}")
            nc.scalar.activation(ow[pb:pb + C_OUT, :wn], ps[pb:pb + C_OUT, :wn],
                                 Ident, bias=cw["b4c"][pb:pb + C_OUT, :])
            nc.scalar.dma_start(out=out_d[0:C_OUT, w * WIN:w * WIN + wn],
                                in_=ow[pb:pb + C_OUT, :wn])

    _insert_library_loads(nc)
    _fix_sync_waits(nc)
    return nc


# ---------------------------------------------------------------- kernel


def prepare(inputs):
    in_feat = np.asarray(inputs["in_feat"], np.float32)
    N = in_feat.shape[0]
    n_local = N // N_CORES
    P_NODES = QUARTERS * _qrows(n_local)
    HALF = P_NODES // 2
    W1 = np.asarray(inputs["W1"], np.float32)
    b1 = np.asarray(inputs["b1"], np.float32)
    W2 = np.asarray(inputs["W2"], np.float32)
    b2 = np.asarray(inputs["b2"], np.float32)
    W3 = np.asarray(inputs["W3"], np.float32)
    b3 = np.asarray(inputs["b3"], np.float32)
    W4 = np.asarray(inputs["W4"], np.float32)
    b4 = np.asarray(inputs["b4"], np.float32)
    srcs = [np.asarray(inputs[f"src{r}"]).astype(np.int64) for r in range(3)]
    dsts = [np.asarray(inputs[f"dst{r}"]).astype(np.int64) for r in range(3)]

    W3a, W3b, W3c = W3[:, 0:H], W3[:, H:2 * H], W3[:, 2 * H:3 * H]
    M = [THETAS[0, k] * W3a + THETAS[1, k] * W3b + THETAS[2, k] * W3c
         for k in range(3)]

    dinvs = []
    for r in range(3):
        deg = np.bincount(dsts[r], minlength=N).astype(np.float32)
        dinvs.append(np.maximum(deg, 1.0) ** -0.5)

    plans = [_plan_relation(srcs[r], dsts[r], N, n_local) for r in range(3)]
    nc = build_nc(plans, n_local)

    def dup(a):
        return np.ascontiguousarray(np.concatenate([a, a], axis=0))

    def dupcol(b):
        col = np.zeros((128, 1), np.float32)
        col[0:len(b), 0] = b
        col[64:64 + len(b), 0] = b
        return col

    iota = np.tile(np.arange(128, dtype=np.int64)[None, :], (128, 1)).astype(np.int8)
    identT = dup(np.eye(H, dtype=np.float32)).astype(BF16)
    in_featT = in_feat.T.copy()

    def pack(a):
        if a.ndim == 1:
            a = np.tile(a[None, :], (H, 1))
        return np.ascontiguousarray(
            np.concatenate([a[:, :HALF], a[:, HALF:]], axis=0))

    in_maps = []
    for c in range(N_CORES):
        m = {
            "in_featT": np.ascontiguousarray(
                np.pad(in_featT[:, c * n_local:(c + 1) * n_local],
                       ((0, 0), (0, P_NODES - n_local)))).astype(BF16),
            "W1T": W1.T.copy().astype(BF16), "W2T": dup(W2.T), "M0T": dup(M[0].T),
            "M1T": dup(M[1].T), "M2T": dup(M[2].T), "W4T": dup(W4.T),
            "b1c": dupcol(b1), "b2c": dupcol(b2), "b3c": dupcol(b3),
            "b4c": dupcol(b4),
            "iota": iota, "identT": identT,
        }
        for r in range(3):
            dl = np.pad(dinvs[r][c * n_local:(c + 1) * n_local],
                        (0, P_NODES - n_local))
            m[f"dinv{r}"] = np.ascontiguousarray(
                np.stack([dl[:HALF], dl[HALF:]]))
            m[f"idx{r}"] = np.ascontiguousarray(plans[r].idx_wrapped[c])
            m[f"dstloc{r}"] = np.ascontiguousarray(plans[r].dstloc[c])
        in_maps.append(m)
    return nc, in_maps, n_local


def kernel(**inputs):
    global LAST_BUILD
    nc, in_maps, n_local = prepare(inputs)
    _lower_library_reloads(nc)
    LAST_BUILD = (nc, in_maps)
    from concourse.bass_utils import run_bass_kernel_spmd
    res = run_bass_kernel_spmd(nc, in_maps, core_ids=list(range(N_CORES)))
    outs = [res.results[c]["out"][:, :n_local] for c in range(N_CORES)]
    return np.ascontiguousarray(np.concatenate(outs, axis=1).T)



# revision 59
# speedup vs baseline: 4.9352x; 1.0897x over previous
"""BWGNN-Hetero forward on 8 Trainium2 NeuronCores.

Node-sharded (N/8 nodes per core). Per relation: two polynomial-propagation
steps; segment-sum gathers per-edge source rows (dma_gather, bf16 tables
with 256B row stride) and reduces them with PE matmuls against
on-device-built one-hot selection matrices into 128-node PSUM windows.

The node table is split into 4 QUARTER tables (one per SWDGE queue): local
rows [3200q, 3200(q+1)) of every core are AllGathered into table_q
[8*3200=25600, 128] (int16-indexable). Gather calls for quarter q run on
SWDGE queue q, so descriptor generation is never ring-stalled behind a
single queue and the 4 rings drain concurrently. Each quarter's AllGather
is triggered as soon as the epilogue finishes that quarter's 25 windows,
overlapping the collective with the remaining descriptor generation.

Node state is feature-major and HALF-PACKED: a [128, P/2] tile holds
features of nodes [0,P/2) on partitions 0:64 and of [P/2,P) on partitions
64:128 (matmuls address the upper half via tile_position).

SPMD: one program for all 8 cores; the edge layout is padded to a common
structure (per-(window,quarter) chunk capacity = max over cores) so the
instruction stream is core-invariant while indices/dst data are inputs.
"""

import os
import numpy as np
import ml_dtypes

import concourse.bass as bass
import concourse.mybir as mybir
import concourse.tile as tile
from concourse import ap_utils
from concourse.bass import MemorySpace

N_CORES = 8
H = 64
C_OUT = 2
IN_F = 128
WIN = 128
CALL_MAX = int(os.environ.get("K_CALLMAX", "1024"))
QUARTERS = 2          # gather tables (== packed partition halves)
NCLS = 2 * QUARTERS   # gather classes: (table, src-node parity)
SPLIT_W = int(os.environ.get("K_SPLITW", "49"))  # windows in AG group 0 (49 = one AG per quarter)
SENT = 1024.0
N_SWDGE_QUEUES = int(os.environ.get("K_QUEUES", "4"))
SKIP_COLL = os.environ.get("K_SKIP_COLL", "0") == "1"
SKIP_GATHER = os.environ.get("K_SKIP_GATHER", "0") == "1"
SKIP_MM = os.environ.get("K_SKIP_MM", "0") == "1"
TINY = os.environ.get("K_TINY", "0") == "1"
TINY_MIN = os.environ.get("K_TINY_MIN", "0") == "1"
MSGP_BUFS = int(os.environ.get("K_MSGP_BUFS", "14"))
HEADSTART = int(os.environ.get("K_HEADSTART", "6"))
SCRATCH = int(os.environ.get("K_SCRATCH", "16384"))


def _qrows(n_local):
    """Padded local rows per quarter (window-aligned)."""
    return -(-n_local // (QUARTERS * WIN)) * WIN

THETAS = np.array([[3.0, -3.0, 0.75],
                   [0.0, 3.0, -1.50],
                   [0.0, 0.0, 0.75]], dtype=np.float32)

BF16 = ml_dtypes.bfloat16
LAST_BUILD = None

# ---------------------------------------------------------------- bir fixes


def _fix_sync_waits(nc, max_waits=1):
    """This walrus build rejects >1 sync-wait per instruction; move excess
    waits onto fresh nops on the same engine queue."""
    counter = [0]
    for fn in nc.m.functions:
        for bb in fn.blocks:
            new_insts = []
            for inst in bb.instructions:
                si = inst.sync_info
                if si is None or not si.on_wait or len(si.on_wait) <= max_waits:
                    new_insts.append(inst)
                    continue
                waits = list(si.on_wait)
                for w in waits[max_waits:]:
                    counter[0] += 1
                    nop = mybir.InstNoOp(name=f"waitsplit_{counter[0]}", ins=[], outs=[])
                    nop.engine = inst.engine
                    nop.sync_info = mybir.SyncInfo(on_wait=[w], on_update=[])
                    nc.register_instruction(nop)
                    new_insts.append(nop)
                inst.sync_info = mybir.SyncInfo(
                    on_wait=waits[:max_waits], on_update=list(si.on_update))
                new_insts.append(inst)
            if len(new_insts) != len(bb.instructions):
                bb.instructions[:] = new_insts


def _insert_library_loads(nc):
    import bass_rust as _bass_rust
    from concourse.library_config import all_libraries, standard
    mask = {}
    for lib in all_libraries:
        for t in lib.instructions:
            mask[t] = mask.get(t, 0) | (1 << lib.index)
    _bass_rust.insert_library_loads(nc, mask, len(all_libraries), standard.index)


def _lower_library_reloads(nc):
    """Rewrite the pseudo library-reload into the raw 64B PSEUDO_INST struct
    this walrus can encode (not sim-executable; call only before HW runs)."""
    import bass_rust as _bass_rust
    isa = nc.isa
    PO = isa.get_enum("NEURON_ISA_TPB_PSEUDO_OPCODE")
    for fn in nc.m.functions:
        for bb in fn.blocks:
            for i, inst in enumerate(bb.instructions):
                if not isinstance(inst, _bass_rust.InstPseudoReloadLibraryIndex):
                    continue
                raw = nc.engines[inst.engine]._isa(
                    isa.Opcode.NEURON_ISA_TPB_OPCODE_PSEUDO_INST,
                    {"pseudo_opcode":
                         PO.NEURON_ISA_TPB_PSEUDO_OPCODE_PSEUDO_LIBRARY_RELOAD_INDEX.value,
                     "lib_index": inst.lib_index},
                    "NEURON_ISA_TPB_PSEUDO_LIBRARY_RELOAD_INDEX_STRUCT",
                    [], [], True)
                raw.engine = inst.engine
                raw.sync_info = inst.sync_info
                nc.register_instruction(raw, overwrite=True)
                bb.instructions[i] = raw


def _dma_gather(gp, out_ap, in_ap, idxs_ap, num_idxs, num_idxs_reg, elem_size,
                elem_step, queue_num=0):
    """dma_gather with the elem_size%256 assert relaxed (row stride must
    still be a 256B multiple; validated on HW)."""
    assert idxs_ap.dtype == mybir.dt.int16
    assert in_ap.dtype == out_ap.dtype
    assert in_ap.space == MemorySpace.DRAM
    assert idxs_ap.space == MemorySpace.SBUF and out_ap.space == MemorySpace.SBUF
    assert ap_utils.ap_is_contiguous(out_ap.ap[1:])
    assert ap_utils.ap_is_contiguous(idxs_ap.ap[1:])
    assert in_ap.ap[-1][1] == out_ap.ap[-1][1] == elem_size
    assert out_ap.ap[0][1] * out_ap.ap[1][1] == ((num_idxs + 127) // 128) * 128
    assert in_ap.ap[0][0] == elem_step
    stride_bytes = elem_step * mybir.dt.size(in_ap.dtype)
    assert stride_bytes % 256 == 0 and stride_bytes // 256 < 256
    _in_ap = gp.lower_ap_dma(in_ap, for_custom_bir_dma=True)
    _idxs_ap = gp.lower_ap(idxs_ap)
    _out_ap = gp.lower_ap(out_ap)
    return gp.add_instruction(
        mybir.InstDMAGatherAnt(
            name=gp.bass.get_next_instruction_name(),
            ins=[*_in_ap, _idxs_ap, gp.lower_val_access(gp.to_reg(num_idxs_reg))],
            outs=[_out_ap],
            transpose=False, num_idxs=num_idxs, elem_size=elem_size,
            stride_bytes_256=stride_bytes // 256, gen_mode=0,
            single_packet=True, queue_num=queue_num, sbuf_tokens_per_rank=0,
            sbuf_free_dim_per_rank=0, sbuf_free_dim_pad_per_rank=0,
            sbuf_byte_offset=0))


# ---------------------------------------------------------------- host plan


def _wrap_idx(idx):
    """[n] -> [128, n/16] int16: idx i at [i%16, i//16], replicated for the
    8 GPSIMD cores across partition groups of 16."""
    n = len(idx)
    assert n % 16 == 0
    w = np.ascontiguousarray(idx.astype(np.int16).reshape(n // 16, 16).T)
    return np.tile(w, (8, 1))


class RelPlan:
    """Common (cross-core) structure + per-core data for one relation."""


def _plan_relation(src, dst, N, n_local):
    n_win = (n_local + WIN - 1) // WIN
    QROWS = _qrows(n_local)

    cores = []
    counts = np.zeros((N_CORES, n_win, NCLS), np.int64)
    for c in range(N_CORES):
        lo = c * n_local
        m = (dst >= lo) & (dst < lo + n_local)
        s = src[m]
        d = dst[m] - lo
        cs = s // n_local
        li = s % n_local
        lq = li % QROWS
        # class = (gather table, src parity); row = pair-row in the table.
        # Each quarter-table is AllGathered in two window-range groups that
        # land in disjoint row ranges: g0 = first SPLIT_W windows (rows
        # [0, 8*G0P)), g1 = the rest (rows [8*G0P, ...)), core-major inside.
        q = (li // QROWS) * 2 + (lq % 2)
        g0n = SPLIT_W * WIN
        g0p = g0n // 2
        g1p = (QROWS - g0n) // 2
        g = (lq >= g0n).astype(np.int64)
        lg = lq - g * g0n
        row = g * (N_CORES * g0p) + cs * np.where(g == 0, g0p, g1p) + lg // 2
        w = d // WIN
        order = np.lexsort((d, w, q))
        row, d, q, w = row[order], d[order], q[order], w[order]
        counts[c] = np.bincount(w * NCLS + q,
                                minlength=n_win * NCLS).reshape(n_win, NCLS)
        cores.append((row, d, q, w))

    cap = np.ceil(counts.max(axis=0) / 128).astype(np.int64)

    chunk_ws = []
    chunk_base = np.zeros((n_win, NCLS), np.int64)
    quarter_first_chunk = []
    g = 0
    for q in range(NCLS):
        quarter_first_chunk.append(g)
        for w in range(n_win):
            chunk_base[w, q] = g
            for _ in range(cap[w, q]):
                chunk_ws.append((w, q))
                g += 1
    n_chunks = g
    n_slots = n_chunks * 128

    # window-aligned calls: a call covers whole windows of one class, so a
    # window's matmuls never straddle two calls of the same class (keeps the
    # msg-pool wait chain linear -> no tile-pool deadlock).
    calls = []
    chunk2call = {}
    max_chunks = CALL_MAX // 128
    for q in range(NCLS):
        w = 0
        while w < n_win:
            c0 = chunk_base[w, q]
            w2 = w
            take = 0
            while w2 < n_win and take + cap[w2, q] <= max_chunks:
                take += cap[w2, q]
                w2 += 1
            assert take > 0
            cid = len(calls)
            calls.append((int(c0) * 128, int(take) * 128, q))
            for j in range(take):
                chunk2call[int(c0) + j] = (cid, j)
            w = w2

    win_chunks = [[] for _ in range(n_win)]
    for gid, (w, q) in enumerate(chunk_ws):
        win_chunks[w].append(gid)
    gid2ipos = np.zeros(max(1, n_chunks), np.int64)
    ipos = 0
    for w in range(n_win):
        for gid in win_chunks[w]:
            gid2ipos[gid] = ipos
            ipos += 1
    call_first_window = [min((chunk_ws[g][0] for g in
                              range(calls[cid][0] // 128,
                                    calls[cid][0] // 128 + calls[cid][1] // 128)),
                             default=0)
                         for cid in range(len(calls))]
    # idx columns are laid out in consumption order so per-call idx loads can
    # be batched into one DMA per group of consecutive calls.
    call_order = sorted(range(len(calls)),
                        key=lambda c: (call_first_window[c], c))
    # Head start: hoist the first HEADSTART class-0/1 calls to the front.
    # Their table half is AllGathered one collective earlier, so their
    # descriptor generation fills the otherwise-idle tail of the previous
    # round (Pool would stall head-of-line on the first class-2/3 call).
    # Bounded so msg-pool allocation cannot deadlock: needs
    # HEADSTART + NCLS <= msgp bufs.
    if HEADSTART > 0:
        early = [c for c in call_order if calls[c][2] < 2][:HEADSTART]
        es = set(early)
        call_order = early + [c for c in call_order if c not in es]
    iw_off = {}
    o = 0
    for cid in call_order:
        iw_off[cid] = o
        o += calls[cid][1] // 16

    idx_data = np.zeros((N_CORES, n_slots), np.int64)
    dst_data = np.full((N_CORES, n_slots), -1.0, np.float64)
    for c in range(N_CORES):
        row, d, q, w = cores[c]
        pos = 0
        for qv in range(NCLS):
            for wv in range(n_win):
                cnt = counts[c, wv, qv]
                if cnt == 0:
                    continue
                b = chunk_base[wv, qv] * 128
                idx_data[c, b:b + cnt] = row[pos:pos + cnt]
                dst_data[c, b:b + cnt] = d[pos:pos + cnt] - wv * WIN
                pos += cnt
        assert pos == len(row)

    p = RelPlan()
    p.n_win = n_win
    p.n_chunks = n_chunks
    p.calls = calls
    p.chunk2call = chunk2call
    p.win_chunks = win_chunks
    p.call_first_window = call_first_window
    p.call_order = call_order
    p.iw_off = iw_off
    p.gid2ipos = gid2ipos
    p.idx_wrapped = [np.concatenate(
        [_wrap_idx(idx_data[c][calls[cid][0]:calls[cid][0] + calls[cid][1]])
         for cid in call_order], axis=1)
        for c in range(N_CORES)]
    dl = np.where(dst_data < 0, -1.0, dst_data)
    p.dstloc = []
    for c in range(N_CORES):
        byg = dl[c].reshape(n_chunks, 128).T
        byi = np.empty_like(byg)
        byi[:, gid2ipos] = byg
        p.dstloc.append(np.ascontiguousarray(byi.astype(np.int8)))
    return p


# ---------------------------------------------------------------- builder


def build_nc(plans, n_local):
    QROWS = _qrows(n_local)
    WPQ = QROWS // WIN
    SHARD_ROWS = N_CORES * (QROWS // 2)   # pair-rows (2 nodes per 256B row)
    assert SHARD_ROWS < 32768
    P_NODES = QUARTERS * QROWS          # 12800
    HALF = P_NODES // 2                 # 6400
    PK = HALF // 128                    # windows per partition-half (50)
    n_win_real = (n_local + 127) // 128  # 98
    fdt = mybir.dt.float32
    bdt = mybir.dt.bfloat16
    iw_cols = [p.idx_wrapped[0].shape[1] for p in plans]
    dl_cols = [p.dstloc[0].shape[1] for p in plans]

    nc = bass.Bass(num_devices=N_CORES, num_swdge_queues=N_SWDGE_QUEUES,
                   dynamic_dma_scratch_size=SCRATCH)
    idt = mybir.dt.int8

    in_featT = (None if TINY_MIN else
                nc.dram_tensor("in_featT", [IN_F, P_NODES], bdt,
                               kind="ExternalInput"))
    wts = {}
    for nm, shape, dt in [
            ("W1T", [IN_F, H], bdt), ("W2T", [128, H], fdt),
            ("M0T", [128, H], fdt), ("M1T", [128, H], fdt), ("M2T", [128, H], fdt),
            ("W4T", [128, C_OUT], fdt),
            ("b1c", [128, 1], fdt), ("b2c", [128, 1], fdt),
            ("b3c", [128, 1], fdt), ("b4c", [128, 1], fdt),
            ("iota", [128, 128], idt), ("identT", [128, H], bdt)]:
        wts[nm] = nc.dram_tensor(nm, shape, dt, kind="ExternalInput")
    if TINY_MIN:
        dinv_d = idx_d = dstloc_d = [None] * 3
    else:
        dinv_d = [nc.dram_tensor(f"dinv{r}", [2, HALF], fdt,
                                 kind="ExternalInput") for r in range(3)]
        idx_d = [nc.dram_tensor(f"idx{r}", [128, iw_cols[r]], mybir.dt.int16,
                                kind="ExternalInput") for r in range(3)]
        dstloc_d = [nc.dram_tensor(f"dstloc{r}", [128, dl_cols[r]], idt,
                                   kind="ExternalInput") for r in range(3)]
    out_d = nc.dram_tensor("out", [C_OUT, P_NODES], fdt, kind="ExternalOutput")

    # per propagation round t (0..5), per table-half q: compact node-major
    # bounce [QROWS, 64] (= [QROWS/2, 128] pair-rows) + AllGathered table
    bounces = [[nc.dram_tensor(f"bounce{t}_{q}", [QROWS, H], bdt)
                for q in range(QUARTERS)] for t in range(6)]
    tables = [[nc.dram_tensor(f"table{t}_{q}", [SHARD_ROWS, 128], bdt,
                              addr_space="Shared")
               for q in range(QUARTERS)] for t in range(6)]
    rg = [list(range(N_CORES))]

    Ident = mybir.ActivationFunctionType.Identity
    Cp = mybir.ActivationFunctionType.Copy

    def wpart(w):
        return 64 * (w // PK), (w % PK) * 128  # (partition base, column base)

    from contextlib import ExitStack
    with tile.TileContext(nc) as tc, ExitStack() as ctx:
        consts = ctx.enter_context(tc.tile_pool(name="consts", bufs=1))
        resid = ctx.enter_context(tc.tile_pool(name="resid", bufs=1))
        hpool = ctx.enter_context(tc.tile_pool(name="hpool", bufs=2))
        f1pool = ctx.enter_context(tc.tile_pool(name="f1pool", bufs=1))
        dinvp = ctx.enter_context(tc.tile_pool(name="dinvp", bufs=2))
        msgp = ctx.enter_context(tc.tile_pool(name="msgp", bufs=MSGP_BUFS))
        selp = ctx.enter_context(tc.tile_pool(name="selp", bufs=2))
        wtile = ctx.enter_context(tc.tile_pool(name="wtile", bufs=2))
        trp = ctx.enter_context(tc.tile_pool(name="trp", bufs=3))
        psum_seg = ctx.enter_context(
            tc.tile_pool(name="psum_seg", bufs=4, space="PSUM"))
        psum_aux = ctx.enter_context(
            tc.tile_pool(name="psum_aux", bufs=2, space="PSUM"))
        psum_tr = ctx.enter_context(
            tc.tile_pool(name="psum_tr", bufs=2, space="PSUM"))
        idxp = ctx.enter_context(tc.tile_pool(name="idxp", bufs=2))
        idxcp = ctx.enter_context(tc.tile_pool(name="idxcp", bufs=8))

        cw = {}
        for nm in wts:
            dt = (idt if nm == "iota"
                  else bdt if nm in ("identT", "W1T") else fdt)
            cw[nm] = consts.tile(list(wts[nm].shape), dt, tag=nm, name=f"cw_{nm}")
            nc.sync.dma_start(out=cw[nm][:], in_=wts[nm][:])
        iota_t = cw["iota"]

        def load_dinv(dst_tile, r):
            for hh in range(2):
                nc.sync.dma_start(
                    out=dst_tile[hh * 64:(hh + 1) * 64, :],
                    in_=dinv_d[r][hh:hh + 1, :].to_broadcast([64, HALF]))

        h_all = resid.tile([128, HALF], fdt)
        nc.vector.memset(h_all[:], 0.0)

        # ---- per-window table build + quarter AllGather trigger
        def build_window_table(w, src_tile, dinv_t, tbl_round):
            pb, cb = wpart(w)
            wn = min(WIN, n_local - w * WIN)
            q = w // WPQ
            scl = wtile.tile([128, WIN], bdt, tag="tblscl", name=f"scl_{tbl_round}_{w}")
            nc.vector.tensor_tensor(out=scl[pb:pb + 64, :wn],
                                    in0=src_tile[pb:pb + 64, cb:cb + wn],
                                    in1=dinv_t[pb:pb + 64, cb:cb + wn],
                                    op=mybir.AluOpType.mult)
            pt = psum_tr.tile([128, H], bdt, tag="tr", name=f"pt_{tbl_round}_{w}")
            nc.tensor.transpose(out=pt[:wn, :], in_=scl[pb:pb + 64, :wn],
                                identity=cw["identT"][pb:pb + 64, :])
            st = trp.tile([128, H], bdt, tag="trs", name=f"trs_{tbl_round}_{w}")
            nc.scalar.activation(st[:wn, :], pt[:wn, :], Cp)
            wq = w - q * WPQ
            nc.scalar.dma_start(
                out=bounces[tbl_round][q][wq * WIN:wq * WIN + wn, 0:H],
                in_=st[:wn, :])
            # Trigger each (quarter, window-group) AllGather a few windows
            # past the group's last window: the wait on the group's bounce
            # DMAs is then on already-drained writes, so the gpsimd gather
            # stream never stalls behind the trigger. (Collectives may only
            # trigger from Pool/DMA engines on trn2.)  The two groups of a
            # quarter land in disjoint row ranges of the same table, keeping
            # the gather-class count (and edge padding) unchanged while the
            # last, blocking AllGather is only half a quarter.
            g0n = SPLIT_W * WIN
            g0p = g0n // 2
            for q2 in range(QUARTERS):
                for g2 in range(2):
                    if g2 == 1 and SPLIT_W == WPQ:
                        continue
                    lg2 = 1 if SPLIT_W < WPQ else 0
                    lw = q2 * WPQ + (SPLIT_W - 1 if g2 == 0 else WPQ - 1)
                    last = (q2 == QUARTERS - 1 and g2 == lg2)
                    tw = lw if last else min(lw + 3, n_win_real - 1)
                    if w == tw and not SKIP_COLL:
                        if g2 == 0:
                            b_ap = bounces[tbl_round][q2][0:g0n, :]
                            t_ap = tables[tbl_round][q2][0:N_CORES * g0p, :]
                        else:
                            b_ap = bounces[tbl_round][q2][g0n:QROWS, :]
                            t_ap = tables[tbl_round][q2][
                                N_CORES * g0p:SHARD_ROWS, :]
                        nc.gpsimd.collective_compute(
                            "AllGather", mybir.AluOpType.bypass,
                            replica_groups=rg,
                            ins=[b_ap.opt()], outs=[t_ap.opt()])

        # ---- initial 2-layer MLP -> h (packed), builds table round 0
        dinv_t = dinvp.tile([128, HALF], fdt, tag="dinv")
        if TINY_MIN:
            nc.vector.memset(dinv_t[:], 0.0)
        else:
            load_dinv(dinv_t, 0)
        # blocked MLP: 4 consecutive windows of one partition-half per op
        # ([64, 512] slices) — same window order as before so the staggered
        # collective triggers in build_window_table stay valid.
        h_cur = hpool.tile([128, HALF], fdt, tag="h")
        MB = 4
        nblk = (PK + MB - 1) // MB
        for hh in range(0 if TINY else 2):
            pb = hh * 64
            for b in range(nblk):
                nb = min(MB, PK - MB * b)
                nw = nb * WIN
                w0 = hh * PK + MB * b
                cb0 = MB * b * WIN
                infw = wtile.tile([IN_F, MB * WIN], bdt, tag="infw",
                                  name=f"infw_{hh}_{b}")
                nc.sync.dma_start(out=infw[:, :nw],
                                  in_=in_featT[:, w0 * WIN:w0 * WIN + nw])
                ps = psum_aux.tile([128, MB * WIN], fdt, tag="aux",
                                   name=f"mlpa_{hh}_{b}")
                for j in range(nb):
                    nc.tensor.matmul(
                        out=ps[pb:pb + 64, j * WIN:(j + 1) * WIN],
                        lhsT=cw["W1T"][:],
                        rhs=infw[:, j * WIN:(j + 1) * WIN],
                        start=True, stop=True, tile_position=(0, pb))
                y1 = wtile.tile([128, MB * WIN], fdt, tag="y1",
                                name=f"y1_{hh}_{b}")
                nc.scalar.activation(y1[pb:pb + 64, :nw], ps[pb:pb + 64, :nw],
                                     Ident, bias=cw["b1c"][pb:pb + 64, :])
                h0w = wtile.tile([128, MB * WIN], fdt, tag="h0w",
                                 name=f"h0w_{hh}_{b}")
                nc.vector.scalar_tensor_tensor(
                    out=h0w[pb:pb + 64, :nw], in0=y1[pb:pb + 64, :nw],
                    scalar=0.01, in1=y1[pb:pb + 64, :nw],
                    op0=mybir.AluOpType.mult, op1=mybir.AluOpType.max)
                ps2 = psum_aux.tile([128, MB * WIN], fdt, tag="aux",
                                    name=f"mlpb_{hh}_{b}")
                for j in range(nb):
                    nc.tensor.matmul(
                        out=ps2[pb:pb + 64, j * WIN:(j + 1) * WIN],
                        lhsT=cw["W2T"][pb:pb + 64, :],
                        rhs=h0w[pb:pb + 64, j * WIN:(j + 1) * WIN],
                        start=True, stop=True, tile_position=(pb, pb))
                y2 = wtile.tile([128, MB * WIN], fdt, tag="y1",
                                name=f"y2_{hh}_{b}")
                nc.scalar.activation(y2[pb:pb + 64, :nw], ps2[pb:pb + 64, :nw],
                                     Ident, bias=cw["b2c"][pb:pb + 64, :])
                nc.vector.scalar_tensor_tensor(
                    out=h_cur[pb:pb + 64, cb0:cb0 + nw],
                    in0=y2[pb:pb + 64, :nw], scalar=0.01,
                    in1=y2[pb:pb + 64, :nw],
                    op0=mybir.AluOpType.mult, op1=mybir.AluOpType.max)
                for j in range(nb):
                    build_window_table(hh * PK + MB * b + j, h_cur, dinv_t, 0)

        nreg_cache = {}

        def propagate(plan, idx_dram, dl_t, tbl_round, epilogue):
            batch = 16
            IB = 8           # gather calls per batched idx DMA
            built = {}
            msg_tiles = {}
            emitted = [0]
            call_order = plan.call_order
            iw_off = plan.iw_off
            ibatch = [None, 0, 0]    # tile, base col, end col

            def emit_calls(up_to_w):
                while emitted[0] < len(call_order):
                    pos = emitted[0]
                    cid = call_order[pos]
                    if pos >= HEADSTART and plan.call_first_window[cid] > up_to_w:
                        break
                    so, n, q = plan.calls[cid]
                    nch = n // 128
                    if iw_off[cid] >= ibatch[2]:
                        c0 = iw_off[cid]
                        c1 = c0
                        for cid2 in call_order[pos:pos + IB]:
                            c1 = iw_off[cid2] + plan.calls[cid2][1] // 16
                        it = idxcp.tile([128, IB * (CALL_MAX // 16)],
                                        mybir.dt.int16, tag="idxc",
                                        name=f"idxc_{tbl_round}_{pos}")
                        nc.sync.dma_start(out=it[:, 0:c1 - c0],
                                          in_=idx_dram[:, c0:c1])
                        ibatch[0], ibatch[1], ibatch[2] = it, c0, c1
                    mt = msgp.tile([128, nch, H], bdt, tag="msg", name=f"msg_{cid}")
                    b0 = iw_off[cid] - ibatch[1]
                    if n not in nreg_cache:
                        nreg_cache[n] = nc.gpsimd.to_reg(n)
                    if not SKIP_GATHER:
                        _dma_gather(
                            nc.gpsimd, out_ap=mt[:],
                            in_ap=tables[tbl_round][q >> 1][
                                0:SHARD_ROWS, (q & 1) * H:(q & 1) * H + H],
                            idxs_ap=ibatch[0][:, b0:b0 + n // 16],
                            num_idxs=n, num_idxs_reg=nreg_cache[n],
                            elem_size=H, elem_step=128, queue_num=q % N_SWDGE_QUEUES)
                    else:
                        nc.vector.memset(mt[:], 0.0)
                    msg_tiles[cid] = mt
                    emitted[0] += 1

            for w in range(plan.n_win):
                emit_calls(w)
                pb, _cb = wpart(w)
                wn = min(WIN, n_local - w * WIN)
                ps = psum_seg.tile([128, 512], fdt, tag="seg", name=f"seg_{w}")
                gids = plan.win_chunks[w]
                if SKIP_MM:
                    gids = []
                if not gids:
                    nc.vector.memset(ps[pb:pb + 64, :wn], 0.0)
                for k, gid in enumerate(gids):
                    ip = int(plan.gid2ipos[gid])
                    bi = ip // batch
                    if bi not in built:
                        i0 = bi * batch
                        nbi = min(batch, plan.n_chunks - i0)
                        st = selp.tile([128, batch * 128], bdt, tag="sel",
                                       name=f"sel_{bi}")
                        nc.vector.tensor_tensor(
                            out=st[:, 0:nbi * 128],
                            in0=dl_t[:, i0:i0 + nbi, None].to_broadcast(
                                [128, nbi, 128]),
                            in1=iota_t[:, None, :].to_broadcast([128, nbi, 128]),
                            op=mybir.AluOpType.is_equal)
                        built[bi] = st
                    st = built[bi]
                    cid, slot = plan.chunk2call[gid]
                    nc.tensor.matmul(
                        out=ps[pb:pb + 64, 0:WIN],
                        lhsT=msg_tiles[cid][:, slot, :],
                        rhs=st[:, (ip - bi * batch) * 128:
                               (ip - bi * batch) * 128 + 128],
                        start=(k == 0), stop=(k == len(gids) - 1),
                        tile_position=(0, pb), skip_group_check=True)
                epilogue(w, ps, wn)

        for r in (() if TINY else range(3)):
            dl_t = idxp.tile([128, dl_cols[r]], idt, tag="dl", name=f"dl_{r}")
            nc.sync.dma_start(out=dl_t[:], in_=dstloc_d[r][:])

            f1 = f1pool.tile([128, HALF], fdt, tag="f1", name=f"f1_{r}")

            # round A (table 2r): produce f1, build table 2r+1 from f1*dinv
            def epi1(w, ps, wn, f1=f1, dinv_t=dinv_t, h_cur=h_cur, r=r):
                pb, cb = wpart(w)
                tmp = wtile.tile([128, WIN], fdt, tag="scaled", name=f"ta_{r}_{w}")
                nc.vector.tensor_tensor(out=tmp[pb:pb + 64, :wn],
                                        in0=ps[pb:pb + 64, :wn],
                                        in1=dinv_t[pb:pb + 64, cb:cb + wn],
                                        op=mybir.AluOpType.mult)
                nc.vector.tensor_tensor(out=f1[pb:pb + 64, cb:cb + wn],
                                        in0=h_cur[pb:pb + 64, cb:cb + wn],
                                        in1=tmp[pb:pb + 64, :wn],
                                        op=mybir.AluOpType.subtract)
                build_window_table(w, f1, dinv_t, 2 * r + 1)

            propagate(plans[r], idx_d[r], dl_t, 2 * r, epi1)

            h_new = hpool.tile([128, HALF], fdt, tag="h", name=f"hn_{r}")
            if r < 2:
                dinv_next = dinvp.tile([128, HALF], fdt, tag="dinv",
                                       name=f"dinv_{r + 1}")
                load_dinv(dinv_next, r + 1)
            else:
                dinv_next = None

            # round B (table 2r+1): produce h_new (+ h_all), build next
            # relation's table 2r+2 from h_new*dinv_{r+1}
            def epi2(w, ps, wn, f1=f1, dinv_t=dinv_t, h_cur=h_cur, h_new=h_new,
                     dinv_next=dinv_next, r=r):
                pb, cb = wpart(w)
                tmp = wtile.tile([128, WIN], fdt, tag="scaled", name=f"tb_{r}_{w}")
                nc.vector.tensor_tensor(out=tmp[pb:pb + 64, :wn],
                                        in0=ps[pb:pb + 64, :wn],
                                        in1=dinv_t[pb:pb + 64, cb:cb + wn],
                                        op=mybir.AluOpType.mult)
                f2w = wtile.tile([128, WIN], fdt, tag="f2w", name=f"f2_{r}_{w}")
                nc.vector.tensor_tensor(out=f2w[pb:pb + 64, :wn],
                                        in0=f1[pb:pb + 64, cb:cb + wn],
                                        in1=tmp[pb:pb + 64, :wn],
                                        op=mybir.AluOpType.subtract)
                ps3 = psum_aux.tile([128, WIN], fdt, tag="aux", name=f"w3_{r}_{w}")
                nc.tensor.matmul(out=ps3[pb:pb + 64, :wn],
                                 lhsT=cw["M0T"][pb:pb + 64, :],
                                 rhs=h_cur[pb:pb + 64, cb:cb + wn],
                                 start=True, stop=False, tile_position=(pb, pb))
                nc.tensor.matmul(out=ps3[pb:pb + 64, :wn],
                                 lhsT=cw["M1T"][pb:pb + 64, :],
                                 rhs=f1[pb:pb + 64, cb:cb + wn],
                                 start=False, stop=False, tile_position=(pb, pb))
                nc.tensor.matmul(out=ps3[pb:pb + 64, :wn],
                                 lhsT=cw["M2T"][pb:pb + 64, :],
                                 rhs=f2w[pb:pb + 64, :wn],
                                 start=False, stop=True, tile_position=(pb, pb))
                nc.scalar.activation(h_new[pb:pb + 64, cb:cb + wn],
                                     ps3[pb:pb + 64, :wn], Ident,
                                     bias=cw["b3c"][pb:pb + 64, :])
                nc.vector.tensor_tensor(out=h_all[pb:pb + 64, cb:cb + wn],
                                        in0=h_all[pb:pb + 64, cb:cb + wn],
                                        in1=h_new[pb:pb + 64, cb:cb + wn],
                                        op=mybir.AluOpType.add)
                if r < 2:
                    build_window_table(w, h_new, dinv_next, 2 * r + 2)

            propagate(plans[r], idx_d[r], dl_t, 2 * r + 1, epi2)
            h_cur = h_new
            if r < 2:
                dinv_t = dinv_next

        # ---- final head (blocked like the MLP)
        for hh in range(2):
            pb = hh * 64
            for b in range(nblk):
                nb = min(MB, PK - MB * b)
                nw = nb * WIN
                w0 = hh * PK + MB * b
                cb0 = MB * b * WIN
                lw = wtile.tile([128, MB * WIN], fdt, tag="lrelu",
                                name=f"lr_{hh}_{b}")
                nc.vector.scalar_tensor_tensor(
                    out=lw[pb:pb + 64, :nw],
                    in0=h_all[pb:pb + 64, cb0:cb0 + nw], scalar=0.01,
                    in1=h_all[pb:pb + 64, cb0:cb0 + nw],
                    op0=mybir.AluOpType.mult, op1=mybir.AluOpType.max)
                ps = psum_aux.tile([128, MB * WIN], fdt, tag="aux",
                                   name=f"hd_{hh}_{b}")
                for j in range(nb):
                    nc.tensor.matmul(
                        out=ps[pb:pb + C_OUT, j * WIN:(j + 1) * WIN],
                        lhsT=cw["W4T"][pb:pb + 64, :],
                        rhs=lw[pb:pb + 64, j * WIN:(j + 1) * WIN],
                        start=True, stop=True, tile_position=(pb, pb))
                ow = trp.tile([128, MB * WIN], fdt, tag="ow",
                              name=f"ow_{hh}_{b}")
                nc.scalar.activation(ow[pb:pb + C_OUT, :nw],
                                     ps[pb:pb + C_OUT, :nw],
                                     Ident, bias=cw["b4c"][pb:pb + C_OUT, :])
                nc.scalar.dma_start(
                    out=out_d[0:C_OUT, w0 * WIN:w0 * WIN + nw],
                    in_=ow[pb:pb + C_OUT, :nw])

    _insert_library_loads(nc)
    _fix_sync_waits(nc)
    return nc


# ---------------------------------------------------------------- kernel


def prepare(inputs):
    in_feat = np.asarray(inputs["in_feat"], np.float32)
    N = in_feat.shape[0]
    n_local = N // N_CORES
    P_NODES = QUARTERS * _qrows(n_local)
    HALF = P_NODES // 2
    W1 = np.asarray(inputs["W1"], np.float32)
    b1 = np.asarray(inputs["b1"], np.float32)
    W2 = np.asarray(inputs["W2"], np.float32)
    b2 = np.asarray(inputs["b2"], np.float32)
    W3 = np.asarray(inputs["W3"], np.float32)
    b3 = np.asarray(inputs["b3"], np.float32)
    W4 = np.asarray(inputs["W4"], np.float32)
    b4 = np.asarray(inputs["b4"], np.float32)
    srcs = [np.asarray(inputs[f"src{r}"]).astype(np.int64) for r in range(3)]
    dsts = [np.asarray(inputs[f"dst{r}"]).astype(np.int64) for r in range(3)]

    W3a, W3b, W3c = W3[:, 0:H], W3[:, H:2 * H], W3[:, 2 * H:3 * H]
    M = [THETAS[0, k] * W3a + THETAS[1, k] * W3b + THETAS[2, k] * W3c
         for k in range(3)]

    dinvs = []
    for r in range(3):
        deg = np.bincount(dsts[r], minlength=N).astype(np.float32)
        dinvs.append(np.maximum(deg, 1.0) ** -0.5)

    plans = [_plan_relation(srcs[r], dsts[r], N, n_local) for r in range(3)]
    nc = build_nc(plans, n_local)

    def dup(a):
        return np.ascontiguousarray(np.concatenate([a, a], axis=0))

    def dupcol(b):
        col = np.zeros((128, 1), np.float32)
        col[0:len(b), 0] = b
        col[64:64 + len(b), 0] = b
        return col

    iota = np.tile(np.arange(128, dtype=np.int64)[None, :], (128, 1)).astype(np.int8)
    identT = dup(np.eye(H, dtype=np.float32)).astype(BF16)
    in_featT = in_feat.T.copy()

    def pack(a):
        if a.ndim == 1:
            a = np.tile(a[None, :], (H, 1))
        return np.ascontiguousarray(
            np.concatenate([a[:, :HALF], a[:, HALF:]], axis=0))

    in_maps = []
    for c in range(N_CORES):
        m = {
            "in_featT": np.ascontiguousarray(
                np.pad(in_featT[:, c * n_local:(c + 1) * n_local],
                       ((0, 0), (0, P_NODES - n_local)))).astype(BF16),
            "W1T": W1.T.copy().astype(BF16), "W2T": dup(W2.T), "M0T": dup(M[0].T),
            "M1T": dup(M[1].T), "M2T": dup(M[2].T), "W4T": dup(W4.T),
            "b1c": dupcol(b1), "b2c": dupcol(b2), "b3c": dupcol(b3),
            "b4c": dupcol(b4),
            "iota": iota, "identT": identT,
        }
        for r in range(3):
            dl = np.pad(dinvs[r][c * n_local:(c + 1) * n_local],
                        (0, P_NODES - n_local))
            m[f"dinv{r}"] = np.ascontiguousarray(
                np.stack([dl[:HALF], dl[HALF:]]))
            m[f"idx{r}"] = np.ascontiguousarray(plans[r].idx_wrapped[c])
            m[f"dstloc{r}"] = np.ascontiguousarray(plans[r].dstloc[c])
        in_maps.append(m)
    return nc, in_maps, n_local


def kernel(**inputs):
    global LAST_BUILD
    nc, in_maps, n_local = prepare(inputs)
    _lower_library_reloads(nc)
    LAST_BUILD = (nc, in_maps)
    from concourse.bass_utils import run_bass_kernel_spmd
    res = run_bass_kernel_spmd(nc, in_maps, core_ids=list(range(N_CORES)))
    outs = [res.results[c]["out"][:, :n_local] for c in range(N_CORES)]
    return np.ascontiguousarray(np.concatenate(outs, axis=1).T)

